# revision 1
# baseline (speedup 1.0000x reference)
"""Self-contained Trainium2 Bass kernel for the 2-layer dual-graph GCN
(nn_GCN0100). Accepts FULL inputs, returns FULL output.

Strategy: node-sharded across 8 NeuronCores, 3 SPMD-style launches:
  run1: h = x @ W1 per shard (fp16 tables)
  run2: layer-1 gather/segment-sum over both graphs (dma_gather + one-hot
        matmul reduction into PSUM), ReLU+bias, h2 = R1 @ W2
  run3: layer-2 gather/segment-sum, logits, log_softmax
Host assembles the full fp16 feature tables between launches (the "halo
exchange") and does index-only graph partitioning; all FLOPs run on device.
"""
import threading
import time
import numpy as np
import jax
import concourse.bass as bass
import concourse.mybir as mybir
import concourse.tile as tile
from concourse import bacc
from concourse.bass2jax import _bass_exec_p, partition_id_tensor, install_neuronx_cc_hook




P = 128
SH = 12800          # shard size (102400 / 8)
NPAD = 102400       # padded node count
CH = 25600          # gather chunk rows (fits int16)
BANK = 512          # PSUM bank slots
STILE = 4096        # S stream SBUF tile free size (fp16 elems per partition)
ITILE = 2048        # idx stream SBUF tile free size (int16 elems per partition)
NIDX_MAX = 6144     # max indices per dma_gather call


def degrees_dinv(edge_index, n=100000):
    deg = np.bincount(np.asarray(edge_index[1]), minlength=n).astype(np.float64) + 1.0
    return (1.0 / np.sqrt(deg)).astype(np.float32)


def build_shard_plan(edge_index, dinv, core):
    """Returns plan dict for one (graph, core) pair."""
    n0 = core * SH
    n1 = n0 + SH
    row = np.asarray(edge_index[0]).astype(np.int64)
    col = np.asarray(edge_index[1]).astype(np.int64)
    m = (col >= n0) & (col < n1)
    row, col = row[m], col[m]
    # self loops for real nodes in shard (nodes >= 100000 are padding)
    selfn = np.arange(n0, min(n1, 100000), dtype=np.int64)
    row = np.concatenate([row, selfn])
    col = np.concatenate([col, selfn])
    norm = (dinv[row] * dinv[col]).astype(np.float32)

    slot = (col - n0).astype(np.int32)
    bank = slot >> 9
    chunk = (row // CH).astype(np.int32)
    lidx = (row % CH).astype(np.int32)

    order = np.lexsort((slot, chunk, bank))
    slot, bank, chunk, lidx, norm = (
        slot[order], bank[order], chunk[order], lidx[order], norm[order]
    )

    nbanks = (SH + BANK - 1) // BANK
    # cell boundaries
    cells = []  # (bank, chunk, idx_arr int32, slot_arr, norm_arr) padded to 128-mult
    key = bank.astype(np.int64) * 8 + chunk
    uniq, starts = np.unique(key, return_index=True)
    starts = np.sort(starts)
    bounds = list(starts) + [len(key)]
    for s, e in zip(bounds[:-1], bounds[1:]):
        b, c = int(bank[s]), int(chunk[s])
        li, sl, nm = lidx[s:e], slot[s:e], norm[s:e]
        pad = (-len(li)) % P
        if pad:
            li = np.concatenate([li, np.full(pad, li[-1], np.int32)])
            sl = np.concatenate([sl, np.full(pad, sl[-1], np.int32)])
            nm = np.concatenate([nm, np.zeros(pad, np.float32)])
        cells.append((b, c, li, sl, nm))

    # gather calls: one call per cell (keeps at most one call live per
    # (bank, chunk) step of the emit loop -> small tile-pool liveness)
    calls = []
    call_of_cell = {}
    for ci, cell in enumerate(cells):
        call_of_cell[ci] = (len(calls), 0)
        calls.append({"chunk": cell[1], "n": len(cell[2]), "idx": cell[2]})

    # windows: per cell, chop into 128-edge windows; emit metadata + S blocks
    windows = []  # (graph-level) dicts: call_id, wslot, bank, smin, B, s_off
    s_blocks = []
    s_off = 0
    for ci, (b, c, li, sl, nm) in enumerate(cells):
        call_id, wbase = call_of_cell[ci]
        nw = len(li) // P
        for w in range(nw):
            ssl = sl[w * P:(w + 1) * P]
            snm = nm[w * P:(w + 1) * P]
            smin = int(ssl.min())
            smax = int(ssl.max())
            B = smax - smin + 1
            S = np.zeros((P, B), np.float16)
            S[np.arange(P), ssl - smin] = snm.astype(np.float16)
            windows.append({
                "call": call_id, "wslot": wbase + w, "bank": b,
                "smin": smin - b * BANK, "B": B, "s_off": s_off,
            })
            s_blocks.append(S)
            s_off += B
    return {
        "cells": cells, "calls": calls, "windows": windows,
        "s_blocks": s_blocks, "nbanks": nbanks,
    }


def pack_streams(plan):
    """Build upload arrays: S stream [128, STOT] fp16 (tile-aligned),
    idx stream [128, ITOT] int16 (call slices tile-aligned, wrapped+replicated),
    and rewrite window/call metadata with tile-local offsets."""
    # S stream
    s_tiles_used = 0
    cur = 0
    offs = []
    for w, S in zip(plan["windows"], plan["s_blocks"]):
        B = w["B"]
        if cur + B > STILE:
            s_tiles_used += 1
            cur = 0
        offs.append((s_tiles_used, cur))
        cur += B
    n_stiles = s_tiles_used + 1
    s_arr = np.zeros((P, n_stiles * STILE), np.float16)
    for (tile_i, off), w, S in zip(offs, plan["windows"], plan["s_blocks"]):
        w["s_tile"] = tile_i
        w["s_col"] = off
        s_arr[:, tile_i * STILE + off: tile_i * STILE + off + w["B"]] = S

    # idx stream: per call, wrapped [16, n/16] replicated to 128 partitions
    i_tiles_used = 0
    cur = 0
    for call in calls_list(plan):
        ncols = call["n"] // 16
        if cur + ncols > ITILE:
            i_tiles_used += 1
            cur = 0
        call["i_tile"] = i_tiles_used
        call["i_col"] = cur
        cur += ncols
    n_itiles = i_tiles_used + 1
    i_arr = np.zeros((P, n_itiles * ITILE), np.int16)
    for call in calls_list(plan):
        idx = call["idx"].astype(np.int16)
        wrapped = idx.reshape(-1, 16).T  # [16, n/16]
        rep = np.tile(wrapped, (8, 1))   # [128, n/16]
        c0 = call["i_tile"] * ITILE + call["i_col"]
        i_arr[:, c0: c0 + wrapped.shape[1]] = rep
    plan["s_arr"] = s_arr
    plan["i_arr"] = i_arr
    plan["n_stiles"] = n_stiles
    plan["n_itiles"] = n_itiles
    return plan


def calls_list(plan):
    return plan["calls"]


# ---------------- numpy emulation of the device algorithm ----------------

def emu_aggregate(plan, table, out_feat):
    """Emulate gathers + window matmuls. table: [NPAD, >=out_feat] fp16.
    Returns aggT [out_feat, SH] float32 (transposed orientation)."""
    nb = plan["nbanks"]
    agg = np.zeros((out_feat, nb * BANK), np.float32)
    gathered = {}
    for cid, call in enumerate(plan["calls"]):
        c = call["chunk"]
        rows = table[c * CH + call["idx"].astype(np.int64)]  # [n, F]
        gathered[cid] = rows
    for w in plan["windows"]:
        g = gathered[w["call"]][w["wslot"] * P:(w["wslot"] + 1) * P, :out_feat]
        S = plan["s_arr"][:, w["s_tile"] * STILE + w["s_col"]:
                          w["s_tile"] * STILE + w["s_col"] + w["B"]]
        # matmul: out[feat, slot] += g[e, feat].T @ S[e, slot]
        contrib = g.astype(np.float32).T @ S.astype(np.float32)
        b0 = w["bank"] * BANK + w["smin"]
        agg[:, b0: b0 + w["B"]] += contrib
    return agg[:, :SH]




F16 = mybir.dt.float16
F32 = mybir.dt.float32
I16 = mybir.dt.int16
NBANK = SH // BANK          # 25
KX = 512 // P               # 4 k-chunks for x@W1


def build_run1():
    """h = x @ W1 for one shard (identical program for all cores).
    Inputs: xT [512, SH] f16, w1 [512, 128] f16. Output: h [SH, 128] f16."""
    nc = bacc.Bacc(None, target_bir_lowering=False)
    xT = nc.dram_tensor("xT", [512, SH], F16, kind="ExternalInput")
    w1 = nc.dram_tensor("w1", [512, 128], F16, kind="ExternalInput")
    h = nc.dram_tensor("h", [SH, 128], F16, kind="ExternalOutput")
    with tile.TileContext(nc) as tc:
        with (
            tc.tile_pool(name="const", bufs=1) as cp,
            tc.tile_pool(name="sb", bufs=3) as sb,
            tc.tile_pool(name="ev", bufs=3) as ev,
            tc.tile_pool(name="ps", bufs=2, space="PSUM") as ps,
        ):
            w1t = cp.tile([128, KX, 128], F16)
            for kc in range(KX):
                nc.sync.dma_start(out=w1t[:, kc, :], in_=w1[kc * 128:(kc + 1) * 128, :])
            for t in range(SH // 512):
                xt = sb.tile([128, KX, 512], F16, tag="xt")
                for kc in range(KX):
                    nc.sync.dma_start(
                        out=xt[:, kc, :],
                        in_=xT[kc * 128:(kc + 1) * 128, t * 512:(t + 1) * 512])
                for s in range(4):
                    pt = ps.tile([128, 128], F32, tag="h")
                    for kc in range(KX):
                        nc.tensor.matmul(
                            out=pt[:], lhsT=xt[:, kc, s * 128:(s + 1) * 128],
                            rhs=w1t[:, kc, :], start=(kc == 0), stop=(kc == KX - 1))
                    he = ev.tile([128, 128], F16, tag="he")
                    nc.vector.tensor_copy(he[:], pt[:])
                    nc.sync.dma_start(
                        out=h[(t * 4 + s) * 128:(t * 4 + s + 1) * 128, :], in_=he[:])
    nc.compile()
    return nc


class AggEmitter:
    """Emits gather calls + window matmuls for one graph, bank at a time."""

    def __init__(self, nc, sb, ps, plan, table, nfeat, tag):
        self.nc, self.sb, self.ps = nc, sb, ps
        self.plan, self.table, self.nfeat, self.tag = plan, table, nfeat, tag
        self.call_tiles = {}
        self.s_tiles = {}
        # windows grouped by bank (plan windows are in (bank, chunk) order)
        self.by_bank = {}
        for w in plan["windows"]:
            self.by_bank.setdefault(w["bank"], []).append(w)

    def _call_tile(self, cid):
        if cid not in self.call_tiles:
            call = self.plan["calls"][cid]
            n = call["n"]
            gt = self.sb.tile([128, n // 128, 128], F16, tag=self.tag + "g")
            it = self.sb.tile([128, n // 16], I16, tag=self.tag + "i")
            c0 = call["i_tile"] * ITILE + call["i_col"]
            self.nc.sync.dma_start(out=it[:], in_=self.plan["dram_i"][:, c0:c0 + n // 16])
            c = call["chunk"]
            self.nc.gpsimd.dma_gather(
                gt[:], self.table[c * CH:(c + 1) * CH, :], it[:], n, n, 128,
                single_packet=False)
            if len(self.call_tiles) > 6:
                for k in sorted(self.call_tiles)[:-5]:
                    del self.call_tiles[k]
            self.call_tiles[cid] = gt
        return self.call_tiles[cid]

    def _s_tile(self, ti):
        if ti not in self.s_tiles:
            st = self.sb.tile([128, STILE], F16, tag=self.tag + "s")
            self.nc.sync.dma_start(
                out=st[:], in_=self.plan["dram_s"][:, ti * STILE:(ti + 1) * STILE])
            if len(self.s_tiles) > 2:
                for k in sorted(self.s_tiles)[:-1]:
                    del self.s_tiles[k]
            self.s_tiles[ti] = st
        return self.s_tiles[ti]

    def emit_bank(self, b):
        """Returns the accumulated PSUM tile [nfeat(pad 128), BANK] for bank b."""
        nc = self.nc
        pt = self.ps.tile([128, BANK], F32, tag=self.tag + "p")
        nc.vector.memset(pt[:self.nfeat, :], 0.0)
        for w in self.by_bank.get(b, []):
            gt = self._call_tile(w["call"])
            st = self._s_tile(w["s_tile"])
            nc.tensor.matmul(
                out=pt[:self.nfeat, w["smin"]:w["smin"] + w["B"]],
                lhsT=gt[:, w["wslot"], :self.nfeat],
                rhs=st[:, w["s_col"]:w["s_col"] + w["B"]],
                start=False, stop=True, skip_group_check=True)
        return pt


def build_run2(plan_s, plan_k):
    """L1 aggregation (both graphs) + R1 + h2 = R1 @ W2 for one core."""
    nc = bacc.Bacc(None, target_bir_lowering=False)
    tb = nc.dram_tensor("tb", [NPAD, 128], F16, kind="ExternalInput")
    sa = nc.dram_tensor("sa", [128, plan_s["n_stiles"] * STILE], F16, kind="ExternalInput")
    ia = nc.dram_tensor("ia", [128, plan_s["n_itiles"] * ITILE], I16, kind="ExternalInput")
    sk = nc.dram_tensor("sk", [128, plan_k["n_stiles"] * STILE], F16, kind="ExternalInput")
    ik = nc.dram_tensor("ik", [128, plan_k["n_itiles"] * ITILE], I16, kind="ExternalInput")
    w2 = nc.dram_tensor("w2", [256, 40], F16, kind="ExternalInput")
    b1v = nc.dram_tensor("b1v", [128, 1], F32, kind="ExternalInput")
    h2 = nc.dram_tensor("h2", [SH, 128], F16, kind="ExternalOutput")
    plan_s["dram_s"], plan_s["dram_i"] = sa, ia
    plan_k["dram_s"], plan_k["dram_i"] = sk, ik
    with tile.TileContext(nc) as tc:
        with (
            tc.tile_pool(name="const", bufs=1) as cp,
            tc.tile_pool(name="sb", bufs=3) as sb,
            tc.tile_pool(name="r1", bufs=2) as r1p,
            tc.tile_pool(name="ev", bufs=3) as ev,
            tc.tile_pool(name="ps", bufs=2, space="PSUM") as ps,
            tc.tile_pool(name="ps2", bufs=2, space="PSUM") as ps2,
        ):
            w2t = cp.tile([128, 2, 40], F16)
            for kc in range(2):
                nc.sync.dma_start(out=w2t[:, kc, :], in_=w2[kc * 128:(kc + 1) * 128, :])
            b1t = cp.tile([128, 1], F32)
            nc.sync.dma_start(out=b1t[:], in_=b1v[:])

            es = AggEmitter(nc, sb, ps, plan_s, tb, 128, "s")
            ek = AggEmitter(nc, sb, ps, plan_k, tb, 128, "k")
            for b in range(NBANK):
                pa = es.emit_bank(b)
                pb = ek.emit_bank(b)
                r1a = r1p.tile([128, BANK], F16, tag="r1a")
                r1b = r1p.tile([128, BANK], F16, tag="r1b")
                nc.scalar.activation(r1a[:], pa[:], mybir.ActivationFunctionType.Relu,
                                     bias=b1t[:, :1], scale=1.0)
                nc.scalar.activation(r1b[:], pb[:], mybir.ActivationFunctionType.Relu,
                                     bias=b1t[:, :1], scale=1.0)
                for s in range(BANK // P):
                    pt = ps2.tile([128, 40], F32, tag="h2")
                    nc.tensor.matmul(out=pt[:], lhsT=r1a[:, s * P:(s + 1) * P],
                                     rhs=w2t[:, 0, :], start=True, stop=False)
                    nc.tensor.matmul(out=pt[:], lhsT=r1b[:, s * P:(s + 1) * P],
                                     rhs=w2t[:, 1, :], start=False, stop=True)
                    he = ev.tile([128, 128], F16, tag="he")
                    nc.vector.memset(he[:], 0.0)
                    nc.vector.tensor_copy(he[:, :40], pt[:])
                    r0 = b * BANK + s * P
                    nc.sync.dma_start(out=h2[r0:r0 + P, :], in_=he[:])
    nc.compile()
    return nc


def build_run3(plan_s, plan_k):
    """L2 aggregation (both graphs) + R2 + logits + log_softmax for one core."""
    nc = bacc.Bacc(None, target_bir_lowering=False)
    tb = nc.dram_tensor("tb", [NPAD, 128], F16, kind="ExternalInput")
    sa = nc.dram_tensor("sa", [128, plan_s["n_stiles"] * STILE], F16, kind="ExternalInput")
    ia = nc.dram_tensor("ia", [128, plan_s["n_itiles"] * ITILE], I16, kind="ExternalInput")
    sk = nc.dram_tensor("sk", [128, plan_k["n_stiles"] * STILE], F16, kind="ExternalInput")
    ik = nc.dram_tensor("ik", [128, plan_k["n_itiles"] * ITILE], I16, kind="ExternalInput")
    wlt = nc.dram_tensor("wlt", [104, 40], F16, kind="ExternalInput")
    b2v = nc.dram_tensor("b2v", [128, 1], F32, kind="ExternalInput")
    blr = nc.dram_tensor("blr", [128, 40], F32, kind="ExternalInput")
    out = nc.dram_tensor("out", [SH, 40], F32, kind="ExternalOutput")
    plan_s["dram_s"], plan_s["dram_i"] = sa, ia
    plan_k["dram_s"], plan_k["dram_i"] = sk, ik
    with tile.TileContext(nc) as tc:
        with (
            tc.tile_pool(name="const", bufs=1) as cp,
            tc.tile_pool(name="sb", bufs=3) as sb,
            tc.tile_pool(name="r2", bufs=2) as r2p,
            tc.tile_pool(name="ev", bufs=4) as ev,
            tc.tile_pool(name="ps", bufs=2, space="PSUM") as ps,
            tc.tile_pool(name="ps2", bufs=2, space="PSUM") as ps2,
        ):
            wltt = cp.tile([104, 40], F16)
            nc.sync.dma_start(out=wltt[:], in_=wlt[:])
            b2t = cp.tile([128, 1], F32)
            nc.sync.dma_start(out=b2t[:], in_=b2v[:])
            blt = cp.tile([128, 40], F32)
            nc.sync.dma_start(out=blt[:], in_=blr[:])

            es = AggEmitter(nc, sb, ps, plan_s, tb, 40, "s")
            ek = AggEmitter(nc, sb, ps, plan_k, tb, 40, "k")
            for b in range(NBANK):
                pa = es.emit_bank(b)
                pb = ek.emit_bank(b)
                r2t = r2p.tile([104, BANK], F16, tag="r2")
                nc.vector.tensor_scalar_add(r2t[0:40, :], pa[:40, :], b2t[:40, :1])
                nc.vector.tensor_scalar_add(r2t[64:104, :], pb[:40, :], b2t[:40, :1])
                for s in range(BANK // P):
                    pt = ps2.tile([128, 40], F32, tag="lg")
                    nc.tensor.matmul(out=pt[:], lhsT=r2t[:, s * P:(s + 1) * P],
                                     rhs=wltt[:], start=True, stop=True)
                    lg = ev.tile([128, 40], F32, tag="lg_sb")
                    nc.vector.tensor_add(lg[:], pt[:], blt[:])
                    mx = ev.tile([128, 1], F32, tag="mx")
                    nc.vector.tensor_reduce(mx[:], lg[:], mybir.AxisListType.X,
                                            mybir.AluOpType.max)
                    mxn = ev.tile([128, 1], F32, tag="mxn")
                    nc.vector.tensor_scalar_mul(mxn[:], mx[:], -1.0)
                    ex = ev.tile([128, 40], F32, tag="ex")
                    sm = ev.tile([128, 1], F32, tag="sm")
                    nc.scalar.activation(ex[:], lg[:], mybir.ActivationFunctionType.Exp,
                                         bias=mxn[:, :1], scale=1.0,
                                         accum_out=sm[:, :1])
                    ls = ev.tile([128, 1], F32, tag="ls")
                    nc.scalar.activation(ls[:], sm[:], mybir.ActivationFunctionType.Ln)
                    c = ev.tile([128, 1], F32, tag="c")
                    nc.vector.tensor_add(c[:], mx[:], ls[:])
                    fin = ev.tile([128, 40], F32, tag="fin")
                    nc.vector.tensor_scalar_sub(fin[:], lg[:], c[:, :1])
                    r0 = b * BANK + s * P
                    nc.sync.dma_start(out=out[r0:r0 + P, :], in_=fin[:])
    nc.compile()
    return nc




class DeviceProgram:
    def __init__(self, nc, device):
        install_neuronx_cc_hook()
        self.nc = nc
        self.device = device
        partition_name = nc.partition_id_tensor.name if nc.partition_id_tensor else None
        in_names, out_names, out_avals, zero_outs = [], [], [], []
        for alloc in nc.m.functions[0].allocations:
            if not isinstance(alloc, mybir.MemoryLocationSet):
                continue
            name = alloc.memorylocations[0].name
            if alloc.kind == "ExternalInput":
                if name != partition_name:
                    in_names.append(name)
            elif alloc.kind == "ExternalOutput":
                shape = tuple(alloc.tensor_shape)
                dtype = mybir.dt.np(alloc.dtype)
                out_names.append(name)
                out_avals.append(jax.core.ShapedArray(shape, dtype))
                zero_outs.append(np.zeros(shape, dtype))
        self.in_names = list(in_names)
        self.out_names = out_names
        self.out_avals = out_avals
        self.zero_outs = zero_outs
        n_params = len(in_names)
        all_names = in_names + out_names + ([partition_name] if partition_name else [])
        self.n_params = n_params
        donate = tuple(range(n_params, n_params + len(out_names)))

        def _body(*args):
            operands = list(args)
            if partition_name is not None:
                operands.append(partition_id_tensor())
            outs = _bass_exec_p.bind(
                *operands,
                out_avals=tuple(out_avals),
                in_names=tuple(all_names),
                out_names=tuple(out_names),
                lowering_input_output_aliases=(),
                sim_require_finite=True,
                sim_require_nnan=True,
                nc=nc,
            )
            return tuple(outs)

        self.fn = jax.jit(_body, donate_argnums=donate, keep_unused=True)
        self.dev_inputs = None

    def upload(self, in_map):
        arrs = [np.asarray(in_map[n]) for n in self.in_names]
        self.dev_inputs = [jax.device_put(a, self.device) for a in arrs]

    def call(self):
        """Run once; returns dict of np outputs. Re-creates donated zero outs."""
        zo = [jax.device_put(z, self.device) for z in self.zero_outs]
        outs = self.fn(*self.dev_inputs, *zo)
        return outs

    def results(self, outs):
        return {n: np.asarray(o) for n, o in zip(self.out_names, outs)}




N_CORES = 8
N_REAL = 100000


def host_prep(edge_index, edge_index_knn):
    dinv_s = degrees_dinv(edge_index)
    dinv_k = degrees_dinv(edge_index_knn)
    plans_s, plans_k = [], []
    for core in range(N_CORES):
        plans_s.append(pack_streams(build_shard_plan(edge_index, dinv_s, core)))
        plans_k.append(pack_streams(build_shard_plan(edge_index_knn, dinv_k, core)))
    return plans_s, plans_k


def build_programs(plans_s, plans_k, verbose=True):
    t0 = time.time()
    nc1 = build_run1()
    if verbose:
        print(f"[build] run1 {time.time()-t0:.1f}s", flush=True)
    nc2s, nc3s = [], []
    for core in range(N_CORES):
        t = time.time()
        nc2s.append(build_run2(plans_s[core], plans_k[core]))
        nc3s.append(build_run3(plans_s[core], plans_k[core]))
        if verbose:
            print(f"[build] core {core} run2+run3 {time.time()-t:.1f}s", flush=True)
    return nc1, nc2s, nc3s


def _parallel(fns):
    outs = [None] * len(fns)
    errs = []

    def wrap(i):
        try:
            outs[i] = fns[i]()
        except Exception as e:  # noqa: BLE001
            import traceback
            errs.append((i, e, traceback.format_exc()))

    ts = [threading.Thread(target=wrap, args=(i,)) for i in range(len(fns))]
    for t in ts:
        t.start()
    for t in ts:
        t.join()
    if errs:
        raise RuntimeError(f"thread errors: {[(i, tb) for i, _, tb in errs]}")
    return outs


class Pipeline:
    def __init__(self, inputs, verbose=True):
        self.v = verbose
        self.inputs = inputs
        self.devices = jax.devices()[:N_CORES]
        t0 = time.time()
        self.plans_s, self.plans_k = host_prep(
            inputs["edge_index"], inputs["edge_index_knn"])
        if verbose:
            print(f"[prep] plans {time.time()-t0:.1f}s", flush=True)
        nc1, nc2s, nc3s = build_programs(self.plans_s, self.plans_k, verbose)
        t0 = time.time()
        self.p1 = [DeviceProgram(nc1, self.devices[i]) for i in range(N_CORES)]
        self.p2 = [DeviceProgram(nc2s[i], self.devices[i]) for i in range(N_CORES)]
        self.p3 = [DeviceProgram(nc3s[i], self.devices[i]) for i in range(N_CORES)]
        if verbose:
            print(f"[build] DevicePrograms {time.time()-t0:.1f}s", flush=True)
        self._prepare_inputs()

    def _prepare_inputs(self):
        ins = self.inputs
        x = np.asarray(ins["x"])
        W1 = np.asarray(ins["W1"]).astype(np.float16)
        W2 = np.asarray(ins["W2"]).astype(np.float16)
        Wlin = np.asarray(ins["Wlin"]).astype(np.float16)
        b1 = np.asarray(ins["b1"]).astype(np.float32)
        b2 = np.asarray(ins["b2"]).astype(np.float32)
        blin = np.asarray(ins["blin"]).astype(np.float32)

        w1p = np.zeros((512, 128), np.float16)
        w1p[:500] = W1
        b1v = b1[:, None]
        b2v = np.zeros((128, 1), np.float32)
        b2v[:40, 0] = b2
        blr = np.tile(blin[None, :], (128, 1)).astype(np.float32)
        wlt = np.zeros((104, 40), np.float16)
        wlt[0:40] = Wlin.T[0:40]
        wlt[64:104] = Wlin.T[40:80]

        self.run1_maps = []
        for i in range(N_CORES):
            xs = np.zeros((SH, 512), np.float16)
            lo, hi = i * SH, min((i + 1) * SH, N_REAL)
            if hi > lo:
                xs[:hi - lo, :500] = x[lo:hi].astype(np.float16)
            self.run1_maps.append({"xT": np.ascontiguousarray(xs.T), "w1": w1p})
        self.consts2 = {"w2": W2, "b1v": b1v}
        self.consts3 = {"wlt": wlt, "b2v": b2v, "blr": blr}

    def run(self, time_it=False):
        v = self.v
        t0 = time.time()
        # ---- run 1
        for i in range(N_CORES):
            self.p1[i].upload(self.run1_maps[i])
        outs1 = _parallel([self.p1[i].call for i in range(N_CORES)])
        h_shards = [self.p1[i].results(outs1[i])["h"] for i in range(N_CORES)]
        table1 = np.concatenate(h_shards, axis=0)  # [NPAD, 128] f16
        if v:
            print(f"[run1] done {time.time()-t0:.1f}s", flush=True)

        # ---- run 2
        t0 = time.time()
        for i in range(N_CORES):
            m = {"tb": table1,
                 "sa": self.plans_s[i]["s_arr"], "ia": self.plans_s[i]["i_arr"],
                 "sk": self.plans_k[i]["s_arr"], "ik": self.plans_k[i]["i_arr"],
                 **self.consts2}
            self.p2[i].upload(m)
        outs2 = _parallel([self.p2[i].call for i in range(N_CORES)])
        h2_shards = [self.p2[i].results(outs2[i])["h2"] for i in range(N_CORES)]
        table2 = np.concatenate(h2_shards, axis=0)  # [NPAD, 128] f16
        if v:
            print(f"[run2] done {time.time()-t0:.1f}s", flush=True)

        # ---- run 3
        t0 = time.time()
        for i in range(N_CORES):
            m = {"tb": table2,
                 "sa": self.plans_s[i]["s_arr"], "ia": self.plans_s[i]["i_arr"],
                 "sk": self.plans_k[i]["s_arr"], "ik": self.plans_k[i]["i_arr"],
                 **self.consts3}
            self.p3[i].upload(m)
        outs3 = _parallel([self.p3[i].call for i in range(N_CORES)])
        out_shards = [self.p3[i].results(outs3[i])["out"] for i in range(N_CORES)]
        result = np.concatenate(out_shards, axis=0)[:N_REAL]
        if v:
            print(f"[run3] done {time.time()-t0:.1f}s", flush=True)

        times = None
        if time_it:
            times = self.time_runs()
        return result, times

    def time_runs(self, reps=5):
        """Concurrent repeat timing per run; returns dict of per-run best wall
        seconds (all 8 devices running concurrently)."""
        times = {}
        for name, progs in (("run1", self.p1), ("run2", self.p2), ("run3", self.p3)):
            best = float("inf")
            for _ in range(reps):
                barrier = threading.Barrier(N_CORES + 1)
                done = []

                def worker(p):
                    barrier.wait()
                    o = p.call()
                    jax.block_until_ready(o)
                    done.append(o)

                ts = [threading.Thread(target=worker, args=(p,)) for p in progs]
                for t in ts:
                    t.start()
                barrier.wait()
                t0 = time.time()
                for t in ts:
                    t.join()
                best = min(best, time.time() - t0)
            times[name] = best
        return times

_PIPELINE_CACHE = {}


def kernel(**inputs):
    key = "singleton"
    pl = _PIPELINE_CACHE.get(key)
    if pl is None or pl.graph_key != _graph_key(inputs):
        pl = Pipeline(inputs, verbose=False)
        pl.graph_key = _graph_key(inputs)
        _PIPELINE_CACHE[key] = pl
    else:
        pl.inputs = inputs
        pl._prepare_inputs()
    out, _ = pl.run(time_it=False)
    return out.astype(np.float32)


def _graph_key(inputs):
    ei = np.asarray(inputs["edge_index"])
    ek = np.asarray(inputs["edge_index_knn"])
    return (ei.shape, ek.shape, int(ei[:, 0].sum()), int(ei[:, -1].sum()),
            int(ek[:, 0].sum()), int(ek[:, -1].sum()))



# revision 11
# speedup vs baseline: 1.2664x; 1.2664x over previous
"""Self-contained Trainium2 Bass kernel for the 2-layer dual-graph GCN
(nn_GCN0100). Accepts FULL inputs, returns FULL output.

Node-sharded across 8 NeuronCores, 3 SPMD-style launches:
  run1: h = x @ W1 per shard (fp16, batched 3D-AP DMAs)
  run2: layer-1 gather/segment-sum over both graphs (dma_gather + fp8
        indicator-matmul into PSUM; dinv_src folded into the tables on the
        host, dinv_col applied on device), ReLU+bias, h2 = R1 @ W2
  run3: layer-2 gather/segment-sum, logits (Wlin split per graph, dinv_col
        as per-partition scalars), log_softmax
Self-loop contributions are added via constant permutation-matrix matmuls
over sequentially-loaded own-shard slabs (this also zero-initializes PSUM).
Host assembles the full fp16 tables between launches (the "halo exchange")
and does index-only graph partitioning + dinv row scaling.
"""
import threading
import time
import numpy as np
import ml_dtypes
import jax
import concourse.bass as bass
import concourse.mybir as mybir
import concourse.tile as tile
from concourse import bacc
from concourse.bass2jax import _bass_exec_p, partition_id_tensor, install_neuronx_cc_hook


P = 128
SH = 12800          # shard size (102400 / 8)
NPAD = 102400       # padded node count
CH = 25600          # gather chunk rows (fits int16)
BANK = 512          # PSUM bank slots
NBANK = SH // BANK  # 25
STILE = 8192        # S stream SBUF tile free size (fp8 elems per partition)
ITILE = 4096        # idx stream SBUF tile free size (int16 elems per partition)
SUBCELL = 2048      # max rows per gather call
N_CORES = 8
N_REAL = 100000

F16 = mybir.dt.float16
F32 = mybir.dt.float32
F8 = mybir.dt.float8e4
I16 = mybir.dt.int16
NPF8 = ml_dtypes.float8_e4m3


def degrees_dinv(edge_index, n=N_REAL):
    deg = np.bincount(np.asarray(edge_index[1]), minlength=n).astype(np.float64) + 1.0
    dinv = (1.0 / np.sqrt(deg)).astype(np.float32)
    return np.concatenate([dinv, np.ones(NPAD - n, np.float32)])


# ---------------------------------------------------------------- host plans

def build_shard_plan(edge_index, core):
    """Per-edge plan (no self-loops): edges sorted by (bank, chunk, slot),
    chopped into <=SUBCELL-row gather calls and 128-row windows with fp8
    indicator S blocks."""
    n0 = core * SH
    n1 = n0 + SH
    row = np.asarray(edge_index[0]).astype(np.int64)
    col = np.asarray(edge_index[1]).astype(np.int64)
    m = (col >= n0) & (col < n1)
    row = row[m].astype(np.int32)
    slot = (col[m] - n0).astype(np.int32)
    bank = slot >> 9
    chunk = row // CH

    o = np.lexsort((slot, chunk, bank))
    bank, chunk, row, slot = bank[o], chunk[o], row[o], slot[o]
    lidx = (row % CH).astype(np.int16)

    # cell boundaries on (bank, chunk)
    key = bank.astype(np.int64) * 4 + chunk
    newcell = np.empty(len(key), bool)
    newcell[0] = True
    newcell[1:] = key[1:] != key[:-1]
    cell_starts = list(np.flatnonzero(newcell)) + [len(key)]

    calls = []
    windows = []
    s_blocks = []
    for c0, c1 in zip(cell_starts[:-1], cell_starts[1:]):
        b = int(bank[c0])
        ck = int(chunk[c0])
        e = c0
        while e < c1:
            e2 = min(e + SUBCELL, c1)
            n = e2 - e
            li = lidx[e:e2]
            pad = (-n) % 16
            if pad:
                li = np.concatenate([li, np.full(pad, li[-1], np.int16)])
            call_id = len(calls)
            calls.append({"chunk": ck, "n": n + pad, "idx": li})
            nw = (n + P - 1) // P
            for w in range(nw):
                a = e + w * P
                bb = min(e + (w + 1) * P, e2)
                ssl = slot[a:bb] - b * BANK
                smin = int(ssl.min())
                B = int(ssl.max()) - smin + 1
                S = np.zeros((P, B), np.float32)
                S[np.arange(bb - a), ssl - smin] = 1.0
                windows.append({"call": call_id, "wslot": w, "bank": b,
                                "smin": smin, "B": B, "rows": bb - a})
                s_blocks.append(S.astype(NPF8))
            e = e2
    return {"calls": calls, "windows": windows, "s_blocks": s_blocks}


def pack_streams(plan):
    tile_i, cur = 0, 0
    for w in plan["windows"]:
        if cur + w["B"] > STILE:
            tile_i += 1
            cur = 0
        w["s_tile"], w["s_col"] = tile_i, cur
        cur += w["B"]
    n_stiles = tile_i + 1
    s_arr = np.zeros((P, n_stiles * STILE), NPF8)
    for w, S in zip(plan["windows"], plan["s_blocks"]):
        c0 = w["s_tile"] * STILE + w["s_col"]
        s_arr[:, c0:c0 + w["B"]] = S

    tile_i, cur = 0, 0
    for call in plan["calls"]:
        ncols = call["n"] // 16
        if cur + ncols > ITILE:
            tile_i += 1
            cur = 0
        call["i_tile"], call["i_col"] = tile_i, cur
        cur += ncols
    n_itiles = tile_i + 1
    i_arr = np.zeros((P, n_itiles * ITILE), np.int16)
    for call in plan["calls"]:
        wrapped = call["idx"].reshape(-1, 16).T
        c0 = call["i_tile"] * ITILE + call["i_col"]
        i_arr[:, c0:c0 + wrapped.shape[1]] = np.tile(wrapped, (8, 1))
    plan["s_arr"] = s_arr
    plan["i_arr"] = i_arr
    plan["n_stiles"] = n_stiles
    plan["n_itiles"] = n_itiles
    return plan


# ------------------------------------------------------------- device: run1

def build_run1():
    """h = x @ W1 for one shard. xT4 [128,4,SH] f16, w14 [128,4,128] f16.
    Out: h4 [128, 100, 128] f16 (h row g*128+p at h4[p, g, :])."""
    nc = bacc.Bacc(None, target_bir_lowering=False)
    xT4 = nc.dram_tensor("xT4", [P, 4, SH], F16, kind="ExternalInput")
    w14 = nc.dram_tensor("w14", [P, 4, P], F16, kind="ExternalInput")
    h4 = nc.dram_tensor("h4", [P, SH // P, P], F16, kind="ExternalOutput")
    with tile.TileContext(nc) as tc:
        with (
            tc.tile_pool(name="const", bufs=1) as cp,
            tc.tile_pool(name="sb", bufs=3) as sb,
            tc.tile_pool(name="ev", bufs=3) as ev,
            tc.tile_pool(name="ps", bufs=2, space="PSUM") as ps,
        ):
            w1t = cp.tile([P, 4, P], F16)
            nc.sync.dma_start(out=w1t[:], in_=w14[:])
            for t in range(SH // 512):
                xt = sb.tile([P, 4, 512], F16, tag="xt")
                nc.sync.dma_start(out=xt[:], in_=xT4[:, :, t * 512:(t + 1) * 512])
                he = ev.tile([P, 4, P], F16, tag="he")
                for s in range(4):
                    pt = ps.tile([P, P], F32, tag="h")
                    for kc in range(4):
                        nc.tensor.matmul(
                            out=pt[:], lhsT=xt[:, kc, s * P:(s + 1) * P],
                            rhs=w1t[:, kc, :], start=(kc == 0), stop=(kc == 3))
                    nc.vector.tensor_copy(he[:, s, :], pt[:])
                nc.sync.dma_start(out=h4[:, t * 4:(t + 1) * 4, :], in_=he[:])
    nc.compile()
    return nc


# --------------------------------------------------------- device: emitters

class AggEmitter:
    """Gather calls + fp8 indicator matmuls for one graph, bank at a time.
    PSUM is initialized by permutation matmuls adding the self-loop
    contribution (transposed own-shard slab)."""

    def __init__(self, nc, pools, plan, table, slab, perm, nfeat, tag):
        self.nc = nc
        self.pg, self.pstream, self.psl, self.ps = pools
        self.plan, self.table, self.nfeat, self.tag = plan, table, nfeat, tag
        self.slab, self.perm = slab, perm
        self.call_tiles = {}
        self.s_tiles = {}
        self.i_tiles = {}
        self.by_bank = {}
        for w in plan["windows"]:
            self.by_bank.setdefault(w["bank"], []).append(w)

    def _i_tile(self, ti):
        if ti not in self.i_tiles:
            it = self.pstream.tile([P, ITILE], I16, tag=self.tag + "i")
            self.nc.sync.dma_start(
                out=it[:], in_=self.plan["dram_i"][:, ti * ITILE:(ti + 1) * ITILE])
            self.i_tiles = {ti: it}
        return self.i_tiles[ti]

    def _call_tile(self, cid):
        if cid not in self.call_tiles:
            call = self.plan["calls"][cid]
            n = call["n"]
            it = self._i_tile(call["i_tile"])
            gt = self.pg.tile([P, (n + P - 1) // P, P], F16, tag=self.tag + "g")
            c = call["chunk"]
            self.nc.gpsimd.dma_gather(
                gt[:], self.table[c * CH:(c + 1) * CH, :],
                it[:, call["i_col"]:call["i_col"] + n // 16], n, n, P,
                single_packet=False)
            self.call_tiles = {cid: gt}
        return self.call_tiles[cid]

    def _s_tile(self, ti):
        if ti not in self.s_tiles:
            st = self.pstream.tile([P, STILE], F8, tag=self.tag + "s")
            self.nc.sync.dma_start(
                out=st[:], in_=self.plan["dram_s"][:, ti * STILE:(ti + 1) * STILE])
            self.s_tiles = {ti: st}
        return self.s_tiles[ti]

    def emit_bank(self, b):
        """PSUM tile [128(nfeat), BANK] = selfT + sum of indicator matmuls."""
        nc = self.nc
        nf = self.nfeat
        pt = self.ps.tile([P, BANK], F32, tag=self.tag + "p")
        # self-loop: transpose own slab rows [b*512, (b+1)*512) via perms
        sl = self.psl.tile([P, 2, 256], F16, tag=self.tag + "sl")
        nc.sync.dma_start(
            out=sl[:], in_=self.slab[:, b * 2:(b + 1) * 2, :])
        for j2 in range(2):
            nc.tensor.matmul(
                out=pt[:nf, j2 * 256:(j2 + 1) * 256],
                lhsT=sl[:, j2, 0:nf], rhs=self.perm[:, 0:256],
                start=True, stop=True, skip_group_check=True)
            nc.tensor.matmul(
                out=pt[:nf, j2 * 256:(j2 + 1) * 256],
                lhsT=sl[:, j2, 128:128 + nf], rhs=self.perm[:, 256:512],
                start=False, stop=True, skip_group_check=True)
        for w in self.by_bank.get(b, []):
            gt = self._call_tile(w["call"])
            st = self._s_tile(w["s_tile"])
            r = w["rows"]
            nc.tensor.matmul(
                out=pt[:nf, w["smin"]:w["smin"] + w["B"]],
                lhsT=gt[:r, w["wslot"], :nf],
                rhs=st[:r, w["s_col"]:w["s_col"] + w["B"]],
                start=False, stop=True, skip_group_check=True)
        return pt


# ------------------------------------------------------------- device: run2

def build_run2(plan_s, plan_k):
    """L1 aggregation (both graphs) + dinv_col postscale + ReLU+b1 +
    h2 = R1 @ W2 for one core. Tables are dinv_src-scaled on host."""
    nc = bacc.Bacc(None, target_bir_lowering=False)
    tbs = nc.dram_tensor("tbs", [NPAD, P], F16, kind="ExternalInput")
    tbk = nc.dram_tensor("tbk", [NPAD, P], F16, kind="ExternalInput")
    sa = nc.dram_tensor("sa", [P, plan_s["n_stiles"] * STILE], F8, kind="ExternalInput")
    ia = nc.dram_tensor("ia", [P, plan_s["n_itiles"] * ITILE], I16, kind="ExternalInput")
    sk = nc.dram_tensor("sk", [P, plan_k["n_stiles"] * STILE], F8, kind="ExternalInput")
    ik = nc.dram_tensor("ik", [P, plan_k["n_itiles"] * ITILE], I16, kind="ExternalInput")
    slbs = nc.dram_tensor("slbs", [P, NBANK * 2, 256], F16, kind="ExternalInput")
    slbk = nc.dram_tensor("slbk", [P, NBANK * 2, 256], F16, kind="ExternalInput")
    prm = nc.dram_tensor("prm", [P, 512], F8, kind="ExternalInput")
    drs = nc.dram_tensor("drs", [1, SH], F16, kind="ExternalInput")
    drk = nc.dram_tensor("drk", [1, SH], F16, kind="ExternalInput")
    w2 = nc.dram_tensor("w2", [P, 2, 40], F16, kind="ExternalInput")
    b1v = nc.dram_tensor("b1v", [P, 1], F32, kind="ExternalInput")
    h24 = nc.dram_tensor("h24", [P, SH // P, 64], F16, kind="ExternalOutput")
    plan_s["dram_s"], plan_s["dram_i"] = sa, ia
    plan_k["dram_s"], plan_k["dram_i"] = sk, ik
    with tile.TileContext(nc) as tc:
        with (
            tc.tile_pool(name="const", bufs=1) as cp,
            tc.tile_pool(name="gt", bufs=4) as pg,
            tc.tile_pool(name="stream", bufs=2) as pstream,
            tc.tile_pool(name="slab", bufs=2) as psl,
            tc.tile_pool(name="r1", bufs=2) as r1p,
            tc.tile_pool(name="ev", bufs=3) as ev,
            tc.tile_pool(name="ps", bufs=2, space="PSUM") as ps,
            tc.tile_pool(name="psb", bufs=1, space="PSUM") as psb,
            tc.tile_pool(name="ps2", bufs=2, space="PSUM") as ps2,
        ):
            w2t = cp.tile([P, 2, 40], F16)
            nc.sync.dma_start(out=w2t[:], in_=w2[:])
            b1t = cp.tile([P, 1], F32)
            nc.sync.dma_start(out=b1t[:], in_=b1v[:])
            prmt = cp.tile([P, 512], F8)
            nc.sync.dma_start(out=prmt[:], in_=prm[:])
            drst = cp.tile([1, SH], F16)
            nc.sync.dma_start(out=drst[:], in_=drs[:])
            drkt = cp.tile([1, SH], F16)
            nc.sync.dma_start(out=drkt[:], in_=drk[:])
            ones = cp.tile([1, P], F16)
            nc.vector.memset(ones[:], 1.0)

            es = AggEmitter(nc, (pg, pstream, psl, ps), plan_s, tbs, slbs, prmt, P, "s")
            ek = AggEmitter(nc, (pg, pstream, psl, ps), plan_k, tbk, slbk, prmt, P, "k")
            for b in range(NBANK):
                pa = es.emit_bank(b)
                pb = ek.emit_bank(b)
                bcs = psb.tile([P, BANK], F32, tag="bcs")
                nc.tensor.matmul(out=bcs[:], lhsT=ones[:],
                                 rhs=drst[:, b * 512:(b + 1) * 512],
                                 start=True, stop=True)
                bck = psb.tile([P, BANK], F32, tag="bck")
                nc.tensor.matmul(out=bck[:], lhsT=ones[:],
                                 rhs=drkt[:, b * 512:(b + 1) * 512],
                                 start=True, stop=True)
                bcss = r1p.tile([P, BANK], F32, tag="bcss")
                bcks = r1p.tile([P, BANK], F32, tag="bcks")
                nc.vector.tensor_copy(bcss[:], bcs[:])
                nc.vector.tensor_copy(bcks[:], bck[:])
                za = r1p.tile([P, BANK], F32, tag="za")
                zb = r1p.tile([P, BANK], F32, tag="zb")
                nc.vector.tensor_mul(za[:], pa[:], bcss[:])
                nc.vector.tensor_mul(zb[:], pb[:], bcks[:])
                r1a = r1p.tile([P, BANK], F16, tag="r1a")
                r1b = r1p.tile([P, BANK], F16, tag="r1b")
                nc.scalar.activation(r1a[:], za[:], mybir.ActivationFunctionType.Relu,
                                     bias=b1t[:, :1], scale=1.0)
                nc.scalar.activation(r1b[:], zb[:], mybir.ActivationFunctionType.Relu,
                                     bias=b1t[:, :1], scale=1.0)
                he = ev.tile([P, 4, 64], F16, tag="he")
                nc.vector.memset(he[:], 0.0)
                for s in range(BANK // P):
                    pt = ps2.tile([P, 40], F32, tag="h2")
                    nc.tensor.matmul(out=pt[:], lhsT=r1a[:, s * P:(s + 1) * P],
                                     rhs=w2t[:, 0, :], start=True, stop=False)
                    nc.tensor.matmul(out=pt[:], lhsT=r1b[:, s * P:(s + 1) * P],
                                     rhs=w2t[:, 1, :], start=False, stop=True)
                    nc.vector.tensor_copy(he[:, s, 0:40], pt[:])
                nc.sync.dma_start(out=h24[:, b * 4:(b + 1) * 4, :], in_=he[:])
    nc.compile()
    return nc


# ------------------------------------------------------------- device: run3

def build_run3(plan_s, plan_k):
    """L2 aggregation (both graphs) + split Wlin matmuls + per-slot dinv_col
    scalars + folded bias + log_softmax for one core."""
    nc = bacc.Bacc(None, target_bir_lowering=False)
    tbs = nc.dram_tensor("tbs", [NPAD, P], F16, kind="ExternalInput")
    tbk = nc.dram_tensor("tbk", [NPAD, P], F16, kind="ExternalInput")
    sa = nc.dram_tensor("sa", [P, plan_s["n_stiles"] * STILE], F8, kind="ExternalInput")
    ia = nc.dram_tensor("ia", [P, plan_s["n_itiles"] * ITILE], I16, kind="ExternalInput")
    sk = nc.dram_tensor("sk", [P, plan_k["n_stiles"] * STILE], F8, kind="ExternalInput")
    ik = nc.dram_tensor("ik", [P, plan_k["n_itiles"] * ITILE], I16, kind="ExternalInput")
    slbs = nc.dram_tensor("slbs", [P, NBANK * 2, 256], F16, kind="ExternalInput")
    slbk = nc.dram_tensor("slbk", [P, NBANK * 2, 256], F16, kind="ExternalInput")
    prm = nc.dram_tensor("prm", [P, 512], F8, kind="ExternalInput")
    wl = nc.dram_tensor("wl", [40, 80], F16, kind="ExternalInput")
    dvs = nc.dram_tensor("dvs", [P, SH // P], F32, kind="ExternalInput")
    dvk = nc.dram_tensor("dvk", [P, SH // P], F32, kind="ExternalInput")
    bf = nc.dram_tensor("bf", [P, 40], F32, kind="ExternalInput")
    out4 = nc.dram_tensor("out4", [P, SH // P, 40], F32, kind="ExternalOutput")
    plan_s["dram_s"], plan_s["dram_i"] = sa, ia
    plan_k["dram_s"], plan_k["dram_i"] = sk, ik
    with tile.TileContext(nc) as tc:
        with (
            tc.tile_pool(name="const", bufs=1) as cp,
            tc.tile_pool(name="gt", bufs=4) as pg,
            tc.tile_pool(name="stream", bufs=2) as pstream,
            tc.tile_pool(name="slab", bufs=2) as psl,
            tc.tile_pool(name="r2", bufs=2) as r2p,
            tc.tile_pool(name="ev", bufs=4) as ev,
            tc.tile_pool(name="ps", bufs=2, space="PSUM") as ps,
            tc.tile_pool(name="ps2", bufs=2, space="PSUM") as ps2,
        ):
            wlt = cp.tile([40, 80], F16)
            nc.sync.dma_start(out=wlt[:], in_=wl[:])
            prmt = cp.tile([P, 512], F8)
            nc.sync.dma_start(out=prmt[:], in_=prm[:])
            dvst = cp.tile([P, SH // P], F32)
            nc.sync.dma_start(out=dvst[:], in_=dvs[:])
            dvkt = cp.tile([P, SH // P], F32)
            nc.sync.dma_start(out=dvkt[:], in_=dvk[:])
            bft = cp.tile([P, 40], F32)
            nc.sync.dma_start(out=bft[:], in_=bf[:])

            es = AggEmitter(nc, (pg, pstream, psl, ps), plan_s, tbs, slbs, prmt, 40, "s")
            ek = AggEmitter(nc, (pg, pstream, psl, ps), plan_k, tbk, slbk, prmt, 40, "k")
            for b in range(NBANK):
                pa = es.emit_bank(b)
                pb = ek.emit_bank(b)
                r2s = r2p.tile([40, BANK], F16, tag="r2s")
                r2k = r2p.tile([40, BANK], F16, tag="r2k")
                nc.vector.tensor_copy(r2s[:], pa[:40, :])
                nc.vector.tensor_copy(r2k[:], pb[:40, :])
                ot = ev.tile([P, 4, 40], F32, tag="ot")
                for s in range(BANK // P):
                    g = b * 4 + s
                    pts = ps2.tile([P, 40], F32, tag="lgs")
                    nc.tensor.matmul(out=pts[:], lhsT=r2s[:, s * P:(s + 1) * P],
                                     rhs=wlt[:, 0:40], start=True, stop=True)
                    ptk = ps2.tile([P, 40], F32, tag="lgk")
                    nc.tensor.matmul(out=ptk[:], lhsT=r2k[:, s * P:(s + 1) * P],
                                     rhs=wlt[:, 40:80], start=True, stop=True)
                    t1 = ev.tile([P, 40], F32, tag="t1")
                    nc.vector.tensor_scalar_mul(t1[:], pts[:], dvst[:, g:g + 1])
                    t2 = ev.tile([P, 40], F32, tag="t2")
                    nc.vector.tensor_scalar_mul(t2[:], ptk[:], dvkt[:, g:g + 1])
                    lg0 = ev.tile([P, 40], F32, tag="lg0")
                    nc.vector.tensor_add(lg0[:], t1[:], t2[:])
                    lg = ev.tile([P, 40], F32, tag="lg")
                    nc.vector.tensor_add(lg[:], lg0[:], bft[:])
                    mx = ev.tile([P, 1], F32, tag="mx")
                    nc.vector.tensor_reduce(mx[:], lg[:], mybir.AxisListType.X,
                                            mybir.AluOpType.max)
                    mxn = ev.tile([P, 1], F32, tag="mxn")
                    nc.vector.tensor_scalar_mul(mxn[:], mx[:], -1.0)
                    exm = ev.tile([P, 40], F32, tag="ex")
                    sm = ev.tile([P, 1], F32, tag="sm")
                    nc.scalar.activation(exm[:], lg[:], mybir.ActivationFunctionType.Exp,
                                         bias=mxn[:, :1], scale=1.0,
                                         accum_out=sm[:, :1])
                    ls = ev.tile([P, 1], F32, tag="ls")
                    nc.scalar.activation(ls[:], sm[:], mybir.ActivationFunctionType.Ln)
                    c = ev.tile([P, 1], F32, tag="c")
                    nc.vector.tensor_add(c[:], mx[:], ls[:])
                    nc.vector.tensor_scalar_sub(ot[:, s, :], lg[:], c[:, :1])
                nc.sync.dma_start(out=out4[:, b * 4:(b + 1) * 4, :], in_=ot[:])
    nc.compile()
    return nc


# ------------------------------------------------------------ device driver

class DeviceProgram:
    def __init__(self, nc, device):
        install_neuronx_cc_hook()
        self.nc = nc
        self.device = device
        partition_name = nc.partition_id_tensor.name if nc.partition_id_tensor else None
        in_names, out_names, out_avals, zero_outs = [], [], [], []
        for alloc in nc.m.functions[0].allocations:
            if not isinstance(alloc, mybir.MemoryLocationSet):
                continue
            name = alloc.memorylocations[0].name
            if alloc.kind == "ExternalInput":
                if name != partition_name:
                    in_names.append(name)
            elif alloc.kind == "ExternalOutput":
                shape = tuple(alloc.tensor_shape)
                dtype = mybir.dt.np(alloc.dtype)
                out_names.append(name)
                out_avals.append(jax.core.ShapedArray(shape, dtype))
                zero_outs.append(np.zeros(shape, dtype))
        self.in_names = list(in_names)
        self.out_names = out_names
        self.out_avals = out_avals
        self.zero_outs = zero_outs
        n_params = len(in_names)
        all_names = in_names + out_names + ([partition_name] if partition_name else [])
        self.n_params = n_params
        donate = tuple(range(n_params, n_params + len(out_names)))

        def _body(*args):
            operands = list(args)
            if partition_name is not None:
                operands.append(partition_id_tensor())
            outs = _bass_exec_p.bind(
                *operands,
                out_avals=tuple(out_avals),
                in_names=tuple(all_names),
                out_names=tuple(out_names),
                lowering_input_output_aliases=(),
                sim_require_finite=True,
                sim_require_nnan=True,
                nc=nc,
            )
            return tuple(outs)

        self.fn = jax.jit(_body, donate_argnums=donate, keep_unused=True)
        self.dev_inputs = None

    def upload(self, in_map):
        arrs = [np.asarray(in_map[n]) for n in self.in_names]
        self.dev_inputs = [jax.device_put(a, self.device) for a in arrs]

    def call(self):
        zo = [jax.device_put(z, self.device) for z in self.zero_outs]
        outs = self.fn(*self.dev_inputs, *zo)
        return outs

    def results(self, outs):
        return {n: np.asarray(o) for n, o in zip(self.out_names, outs)}


def _parallel(fns):
    outs = [None] * len(fns)
    errs = []

    def wrap(i):
        try:
            outs[i] = fns[i]()
        except Exception as e:  # noqa: BLE001
            import traceback
            errs.append((i, e, traceback.format_exc()))

    ts = [threading.Thread(target=wrap, args=(i,)) for i in range(len(fns))]
    for t in ts:
        t.start()
    for t in ts:
        t.join()
    if errs:
        raise RuntimeError(f"thread errors: {[(i, tb) for i, _, tb in errs]}")
    return outs


# ------------------------------------------------------------------ pipeline

def host_prep(edge_index, edge_index_knn):
    plans_s, plans_k = [], []
    for core in range(N_CORES):
        plans_s.append(pack_streams(build_shard_plan(edge_index, core)))
        plans_k.append(pack_streams(build_shard_plan(edge_index_knn, core)))
    return plans_s, plans_k


def build_programs(plans_s, plans_k, verbose=True):
    t0 = time.time()
    nc1 = build_run1()
    if verbose:
        print(f"[build] run1 {time.time()-t0:.1f}s", flush=True)
    nc2s, nc3s = [], []
    for core in range(N_CORES):
        t = time.time()
        nc2s.append(build_run2(plans_s[core], plans_k[core]))
        nc3s.append(build_run3(plans_s[core], plans_k[core]))
        if verbose:
            print(f"[build] core {core} run2+run3 {time.time()-t:.1f}s", flush=True)
    return nc1, nc2s, nc3s


def make_perm():
    pe = np.zeros((P, 512), NPF8)
    for i in range(P):
        pe[i, 2 * i] = 1.0          # P_even: row i -> col 2i
        pe[i, 256 + 2 * i + 1] = 1.0  # P_odd: row i -> col 2i+1
    return pe


def slab_of(tb, core):
    """Own-shard slab [128, NBANK*2, 256]: block j covers rows
    n0+j*256 .. +255; partition p holds rows (j*256+2p, j*256+2p+1)."""
    n0 = core * SH
    sl = tb[n0:n0 + SH].reshape(NBANK * 2, P, 256).transpose(1, 0, 2)
    return np.ascontiguousarray(sl)


class Pipeline:
    def __init__(self, inputs, verbose=True):
        self.v = verbose
        self.inputs = inputs
        self.devices = jax.devices()[:N_CORES]
        t0 = time.time()
        self.plans_s, self.plans_k = host_prep(
            inputs["edge_index"], inputs["edge_index_knn"])
        self.dinv_s = degrees_dinv(inputs["edge_index"])
        self.dinv_k = degrees_dinv(inputs["edge_index_knn"])
        if verbose:
            print(f"[prep] plans {time.time()-t0:.1f}s", flush=True)
        nc1, nc2s, nc3s = build_programs(self.plans_s, self.plans_k, verbose)
        t0 = time.time()
        self.p1 = [DeviceProgram(nc1, self.devices[i]) for i in range(N_CORES)]
        self.p2 = [DeviceProgram(nc2s[i], self.devices[i]) for i in range(N_CORES)]
        self.p3 = [DeviceProgram(nc3s[i], self.devices[i]) for i in range(N_CORES)]
        if verbose:
            print(f"[build] DevicePrograms {time.time()-t0:.1f}s", flush=True)
        self._prepare_inputs()

    def _prepare_inputs(self):
        ins = self.inputs
        x = np.asarray(ins["x"])
        W1 = np.asarray(ins["W1"]).astype(np.float16)
        W2 = np.asarray(ins["W2"]).astype(np.float16)
        Wlin = np.asarray(ins["Wlin"]).astype(np.float32)
        b1 = np.asarray(ins["b1"]).astype(np.float32)
        b2 = np.asarray(ins["b2"]).astype(np.float32)
        blin = np.asarray(ins["blin"]).astype(np.float32)

        w1p = np.zeros((512, P), np.float16)
        w1p[:500] = W1
        w14 = np.ascontiguousarray(w1p.reshape(4, P, P).transpose(1, 0, 2))
        self.run1_maps = []
        for i in range(N_CORES):
            xs = np.zeros((SH, 512), np.float16)
            lo, hi = i * SH, min((i + 1) * SH, N_REAL)
            if hi > lo:
                xs[:hi - lo, :500] = x[lo:hi].astype(np.float16)
            xT4 = np.ascontiguousarray(xs.T.reshape(4, P, SH).transpose(1, 0, 2))
            self.run1_maps.append({"xT4": xT4, "w14": w14})

        w2t = np.ascontiguousarray(
            np.asarray(W2).reshape(2, P, 40).transpose(1, 0, 2))
        self.consts2 = {"w2": w2t, "b1v": b1[:, None], "prm": make_perm()}
        WlT = Wlin.T  # [80, 40]
        wl = np.zeros((40, 80), np.float16)
        wl[:, 0:40] = WlT[0:40]
        wl[:, 40:80] = WlT[40:80]
        biasf = b2 @ (WlT[0:40] + WlT[40:80]).astype(np.float64) + blin
        bft = np.tile(biasf.astype(np.float32)[None, :], (P, 1))
        self.consts3 = {"wl": wl, "bf": bft, "prm": make_perm()}

        self.dr_s = [np.ascontiguousarray(
            self.dinv_s[i * SH:(i + 1) * SH].astype(np.float16).reshape(1, SH))
            for i in range(N_CORES)]
        self.dr_k = [np.ascontiguousarray(
            self.dinv_k[i * SH:(i + 1) * SH].astype(np.float16).reshape(1, SH))
            for i in range(N_CORES)]
        # per-slot-group [128, 100] fp32: dvs[p, g] = dinv[n0 + g*128 + p]
        self.dv_s = [np.ascontiguousarray(
            self.dinv_s[i * SH:(i + 1) * SH].reshape(SH // P, P).T)
            for i in range(N_CORES)]
        self.dv_k = [np.ascontiguousarray(
            self.dinv_k[i * SH:(i + 1) * SH].reshape(SH // P, P).T)
            for i in range(N_CORES)]

    def run(self):
        v = self.v
        t0 = time.time()
        for i in range(N_CORES):
            self.p1[i].upload(self.run1_maps[i])
        outs1 = _parallel([self.p1[i].call for i in range(N_CORES)])
        h_shards = [self.p1[i].results(outs1[i])["h4"] for i in range(N_CORES)]
        h_all = np.concatenate(
            [h.transpose(1, 0, 2).reshape(SH, P) for h in h_shards], axis=0)
        tb_s = (h_all.astype(np.float32) * self.dinv_s[:, None]).astype(np.float16)
        tb_k = (h_all.astype(np.float32) * self.dinv_k[:, None]).astype(np.float16)
        if v:
            print(f"[run1] done {time.time()-t0:.1f}s", flush=True)

        t0 = time.time()
        for i in range(N_CORES):
            m = {"tbs": tb_s, "tbk": tb_k,
                 "slbs": slab_of(tb_s, i), "slbk": slab_of(tb_k, i),
                 "sa": self.plans_s[i]["s_arr"], "ia": self.plans_s[i]["i_arr"],
                 "sk": self.plans_k[i]["s_arr"], "ik": self.plans_k[i]["i_arr"],
                 "drs": self.dr_s[i], "drk": self.dr_k[i],
                 **self.consts2}
            self.p2[i].upload(m)
        outs2 = _parallel([self.p2[i].call for i in range(N_CORES)])
        h2_shards = [self.p2[i].results(outs2[i])["h24"] for i in range(N_CORES)]
        h2_all = np.concatenate(
            [h.transpose(1, 0, 2).reshape(SH, 64) for h in h2_shards], axis=0)
        tb2_s = np.zeros((NPAD, P), np.float16)
        tb2_k = np.zeros((NPAD, P), np.float16)
        h2f = h2_all[:N_REAL, :40].astype(np.float32)
        tb2_s[:N_REAL, :40] = (h2f * self.dinv_s[:N_REAL, None]).astype(np.float16)
        tb2_k[:N_REAL, :40] = (h2f * self.dinv_k[:N_REAL, None]).astype(np.float16)
        if v:
            print(f"[run2] done {time.time()-t0:.1f}s", flush=True)

        t0 = time.time()
        for i in range(N_CORES):
            m = {"tbs": tb2_s, "tbk": tb2_k,
                 "slbs": slab_of(tb2_s, i), "slbk": slab_of(tb2_k, i),
                 "sa": self.plans_s[i]["s_arr"], "ia": self.plans_s[i]["i_arr"],
                 "sk": self.plans_k[i]["s_arr"], "ik": self.plans_k[i]["i_arr"],
                 "dvs": self.dv_s[i], "dvk": self.dv_k[i],
                 **self.consts3}
            self.p3[i].upload(m)
        outs3 = _parallel([self.p3[i].call for i in range(N_CORES)])
        out_shards = [self.p3[i].results(outs3[i])["out4"] for i in range(N_CORES)]
        result = np.concatenate(
            [o.transpose(1, 0, 2).reshape(SH, 40) for o in out_shards], axis=0)
        if v:
            print(f"[run3] done {time.time()-t0:.1f}s", flush=True)
        return result[:N_REAL]

    def time_runs(self, reps=5):
        times = {}
        for name, progs in (("run1", self.p1), ("run2", self.p2), ("run3", self.p3)):
            best = float("inf")
            for _ in range(reps):
                barrier = threading.Barrier(N_CORES + 1)
                done = []

                def worker(p):
                    barrier.wait()
                    o = p.call()
                    jax.block_until_ready(o)
                    done.append(o)

                ts = [threading.Thread(target=worker, args=(p,)) for p in progs]
                for t in ts:
                    t.start()
                barrier.wait()
                t0 = time.time()
                for t in ts:
                    t.join()
                best = min(best, time.time() - t0)
            times[name] = best
        return times


_PIPELINE_CACHE = {}


def _graph_key(inputs):
    ei = np.asarray(inputs["edge_index"])
    ek = np.asarray(inputs["edge_index_knn"])
    return (ei.shape, ek.shape, int(ei[:, 0].sum()), int(ei[:, -1].sum()),
            int(ek[:, 0].sum()), int(ek[:, -1].sum()))


def kernel(**inputs):
    key = "singleton"
    pl = _PIPELINE_CACHE.get(key)
    if pl is None or pl.graph_key != _graph_key(inputs):
        pl = Pipeline(inputs, verbose=False)
        pl.graph_key = _graph_key(inputs)
        _PIPELINE_CACHE[key] = pl
    else:
        pl.inputs = inputs
        pl._prepare_inputs()
    out = pl.run()
    return out.astype(np.float32)


# revision 12
# speedup vs baseline: 1.2692x; 1.0022x over previous
"""Self-contained Trainium2 Bass kernel for the 2-layer dual-graph GCN
(nn_GCN0100). Accepts FULL inputs, returns FULL output.

Node-sharded across 8 NeuronCores, 3 SPMD-style launches:
  run1: h = x @ W1 per shard (fp16, batched 3D-AP DMAs)
  run2: layer-1 gather/segment-sum over both graphs (dma_gather + fp8
        indicator-matmul into PSUM; dinv_src folded into the tables on the
        host, dinv_col applied on device), ReLU+bias, h2 = R1 @ W2
  run3: layer-2 gather/segment-sum, logits (Wlin split per graph, dinv_col
        as per-partition scalars), log_softmax
Self-loop contributions are added via constant permutation-matrix matmuls
over sequentially-loaded own-shard slabs (this also zero-initializes PSUM).
Host assembles the full fp16 tables between launches (the "halo exchange")
and does index-only graph partitioning + dinv row scaling.
"""
import threading
import time
import numpy as np
import ml_dtypes
import jax
import concourse.bass as bass
import concourse.mybir as mybir
import concourse.tile as tile
from concourse import bacc
from concourse.bass2jax import _bass_exec_p, partition_id_tensor, install_neuronx_cc_hook


P = 128
SH = 12800          # shard size (102400 / 8)
NPAD = 102400       # padded node count
CH = 25600          # gather chunk rows (fits int16)
BANK = 512          # PSUM bank slots
NBANK = SH // BANK  # 25
STILE = 4096        # S stream SBUF tile free size (fp8 elems per partition)
ITILE = 4096        # idx stream SBUF tile free size (int16 elems per partition)
SUBCELL = 2048      # max rows per gather call
N_CORES = 8
N_REAL = 100000

F16 = mybir.dt.float16
F32 = mybir.dt.float32
F8 = mybir.dt.float8e4
I16 = mybir.dt.int16
NPF8 = ml_dtypes.float8_e4m3


def degrees_dinv(edge_index, n=N_REAL):
    deg = np.bincount(np.asarray(edge_index[1]), minlength=n).astype(np.float64) + 1.0
    dinv = (1.0 / np.sqrt(deg)).astype(np.float32)
    return np.concatenate([dinv, np.ones(NPAD - n, np.float32)])


# ---------------------------------------------------------------- host plans

def build_shard_plan(edge_index, core):
    """Per-edge plan (no self-loops): edges sorted by (bank, chunk, slot),
    chopped into <=SUBCELL-row gather calls and 128-row windows with fp8
    indicator S blocks."""
    n0 = core * SH
    n1 = n0 + SH
    row = np.asarray(edge_index[0]).astype(np.int64)
    col = np.asarray(edge_index[1]).astype(np.int64)
    m = (col >= n0) & (col < n1)
    row = row[m].astype(np.int32)
    slot = (col[m] - n0).astype(np.int32)
    bank = slot >> 9
    chunk = row // CH

    o = np.lexsort((slot, chunk, bank))
    bank, chunk, row, slot = bank[o], chunk[o], row[o], slot[o]
    lidx = (row % CH).astype(np.int16)

    # cell boundaries on (bank, chunk)
    key = bank.astype(np.int64) * 4 + chunk
    newcell = np.empty(len(key), bool)
    newcell[0] = True
    newcell[1:] = key[1:] != key[:-1]
    cell_starts = list(np.flatnonzero(newcell)) + [len(key)]

    calls = []
    windows = []
    s_blocks = []
    for c0, c1 in zip(cell_starts[:-1], cell_starts[1:]):
        b = int(bank[c0])
        ck = int(chunk[c0])
        e = c0
        while e < c1:
            e2 = min(e + SUBCELL, c1)
            n = e2 - e
            li = lidx[e:e2]
            pad = (-n) % 16
            if pad:
                li = np.concatenate([li, np.full(pad, li[-1], np.int16)])
            call_id = len(calls)
            calls.append({"chunk": ck, "n": n + pad, "idx": li})
            nw = (n + P - 1) // P
            for w in range(nw):
                a = e + w * P
                bb = min(e + (w + 1) * P, e2)
                ssl = slot[a:bb] - b * BANK
                smin = int(ssl.min())
                B = int(ssl.max()) - smin + 1
                S = np.zeros((P, B), np.float32)
                S[np.arange(bb - a), ssl - smin] = 1.0
                windows.append({"call": call_id, "wslot": w, "bank": b,
                                "smin": smin, "B": B, "rows": bb - a})
                s_blocks.append(S.astype(NPF8))
            e = e2
    return {"calls": calls, "windows": windows, "s_blocks": s_blocks}


def pack_streams(plan):
    tile_i, cur = 0, 0
    for w in plan["windows"]:
        if cur + w["B"] > STILE:
            tile_i += 1
            cur = 0
        w["s_tile"], w["s_col"] = tile_i, cur
        cur += w["B"]
    n_stiles = tile_i + 1
    s_arr = np.zeros((P, n_stiles * STILE), NPF8)
    for w, S in zip(plan["windows"], plan["s_blocks"]):
        c0 = w["s_tile"] * STILE + w["s_col"]
        s_arr[:, c0:c0 + w["B"]] = S

    tile_i, cur = 0, 0
    for call in plan["calls"]:
        ncols = call["n"] // 16
        if cur + ncols > ITILE:
            tile_i += 1
            cur = 0
        call["i_tile"], call["i_col"] = tile_i, cur
        cur += ncols
    n_itiles = tile_i + 1
    i_arr = np.zeros((P, n_itiles * ITILE), np.int16)
    for call in plan["calls"]:
        wrapped = call["idx"].reshape(-1, 16).T
        c0 = call["i_tile"] * ITILE + call["i_col"]
        i_arr[:, c0:c0 + wrapped.shape[1]] = np.tile(wrapped, (8, 1))
    plan["s_arr"] = s_arr
    plan["i_arr"] = i_arr
    plan["n_stiles"] = n_stiles
    plan["n_itiles"] = n_itiles
    return plan


# ------------------------------------------------------------- device: run1

def build_run1():
    """h = x @ W1 for one shard. xT4 [128,4,SH] f16, w14 [128,4,128] f16.
    Out: h4 [128, 100, 128] f16 (h row g*128+p at h4[p, g, :])."""
    nc = bacc.Bacc(None, target_bir_lowering=False)
    xT4 = nc.dram_tensor("xT4", [P, 4, SH], F16, kind="ExternalInput")
    w14 = nc.dram_tensor("w14", [P, 4, P], F16, kind="ExternalInput")
    h4 = nc.dram_tensor("h4", [P, SH // P, P], F16, kind="ExternalOutput")
    with tile.TileContext(nc) as tc:
        with (
            tc.tile_pool(name="const", bufs=1) as cp,
            tc.tile_pool(name="sb", bufs=3) as sb,
            tc.tile_pool(name="ev", bufs=3) as ev,
            tc.tile_pool(name="ps", bufs=2, space="PSUM") as ps,
        ):
            w1t = cp.tile([P, 4, P], F16)
            nc.sync.dma_start(out=w1t[:], in_=w14[:])
            for t in range(SH // 512):
                xt = sb.tile([P, 4, 512], F16, tag="xt")
                nc.sync.dma_start(out=xt[:], in_=xT4[:, :, t * 512:(t + 1) * 512])
                he = ev.tile([P, 4, P], F16, tag="he")
                for s in range(4):
                    pt = ps.tile([P, P], F32, tag="h")
                    for kc in range(4):
                        nc.tensor.matmul(
                            out=pt[:], lhsT=xt[:, kc, s * P:(s + 1) * P],
                            rhs=w1t[:, kc, :], start=(kc == 0), stop=(kc == 3))
                    nc.vector.tensor_copy(he[:, s, :], pt[:])
                nc.sync.dma_start(out=h4[:, t * 4:(t + 1) * 4, :], in_=he[:])
    nc.compile()
    return nc


# --------------------------------------------------------- device: emitters

class AggEmitter:
    """Gather calls + fp8 indicator matmuls for one graph, bank at a time.
    PSUM is initialized by permutation matmuls adding the self-loop
    contribution (transposed own-shard slab)."""

    def __init__(self, nc, pools, plan, table, slab, perm, nfeat, tag):
        self.nc = nc
        self.pg, self.pstream, self.psl, self.ps = pools
        self.plan, self.table, self.nfeat, self.tag = plan, table, nfeat, tag
        self.slab, self.perm = slab, perm
        self.call_tiles = {}
        self.s_tiles = {}
        self.i_tiles = {}
        self.by_bank = {}
        for w in plan["windows"]:
            self.by_bank.setdefault(w["bank"], []).append(w)

    def _i_tile(self, ti):
        if ti not in self.i_tiles:
            it = self.pstream.tile([P, ITILE], I16, tag=self.tag + "i")
            self.nc.sync.dma_start(
                out=it[:], in_=self.plan["dram_i"][:, ti * ITILE:(ti + 1) * ITILE])
            self.i_tiles = {ti: it}
        return self.i_tiles[ti]

    def _call_tile(self, cid):
        if cid not in self.call_tiles:
            call = self.plan["calls"][cid]
            n = call["n"]
            it = self._i_tile(call["i_tile"])
            gt = self.pg.tile([P, (n + P - 1) // P, P], F16, tag=self.tag + "g")
            c = call["chunk"]
            self.nc.gpsimd.dma_gather(
                gt[:], self.table[c * CH:(c + 1) * CH, :],
                it[:, call["i_col"]:call["i_col"] + n // 16], n, n, P,
                single_packet=False)
            self.call_tiles = {cid: gt}
        return self.call_tiles[cid]

    def _s_tile(self, ti):
        if ti not in self.s_tiles:
            st8 = self.pstream.tile([P, STILE], F8, tag=self.tag + "s8")
            self.nc.sync.dma_start(
                out=st8[:], in_=self.plan["dram_s"][:, ti * STILE:(ti + 1) * STILE])
            st = self.pstream.tile([P, STILE], F16, tag=self.tag + "s")
            self.nc.vector.tensor_copy(st[:], st8[:])
            self.s_tiles = {ti: st}
        return self.s_tiles[ti]

    def emit_bank(self, b):
        """PSUM tile [128(nfeat), BANK] = selfT + sum of indicator matmuls."""
        nc = self.nc
        nf = self.nfeat
        pt = self.ps.tile([P, BANK], F32, tag=self.tag + "p")
        # self-loop: transpose own slab rows [b*512, (b+1)*512) via perms
        sl = self.psl.tile([P, 2, 256], F16, tag=self.tag + "sl")
        nc.sync.dma_start(
            out=sl[:], in_=self.slab[:, b * 2:(b + 1) * 2, :])
        for j2 in range(2):
            nc.tensor.matmul(
                out=pt[:nf, j2 * 256:(j2 + 1) * 256],
                lhsT=sl[:, j2, 0:nf], rhs=self.perm[:, 0:256],
                start=True, stop=True, skip_group_check=True)
            nc.tensor.matmul(
                out=pt[:nf, j2 * 256:(j2 + 1) * 256],
                lhsT=sl[:, j2, 128:128 + nf], rhs=self.perm[:, 256:512],
                start=False, stop=True, skip_group_check=True)
        for w in self.by_bank.get(b, []):
            gt = self._call_tile(w["call"])
            st = self._s_tile(w["s_tile"])
            r = w["rows"]
            nc.tensor.matmul(
                out=pt[:nf, w["smin"]:w["smin"] + w["B"]],
                lhsT=gt[:r, w["wslot"], :nf],
                rhs=st[:r, w["s_col"]:w["s_col"] + w["B"]],
                start=False, stop=True, skip_group_check=True)
        return pt


# ------------------------------------------------------------- device: run2

def build_run2(plan_s, plan_k):
    """L1 aggregation (both graphs) + dinv_col postscale + ReLU+b1 +
    h2 = R1 @ W2 for one core. Tables are dinv_src-scaled on host."""
    nc = bacc.Bacc(None, target_bir_lowering=False)
    tbs = nc.dram_tensor("tbs", [NPAD, P], F16, kind="ExternalInput")
    tbk = nc.dram_tensor("tbk", [NPAD, P], F16, kind="ExternalInput")
    sa = nc.dram_tensor("sa", [P, plan_s["n_stiles"] * STILE], F8, kind="ExternalInput")
    ia = nc.dram_tensor("ia", [P, plan_s["n_itiles"] * ITILE], I16, kind="ExternalInput")
    sk = nc.dram_tensor("sk", [P, plan_k["n_stiles"] * STILE], F8, kind="ExternalInput")
    ik = nc.dram_tensor("ik", [P, plan_k["n_itiles"] * ITILE], I16, kind="ExternalInput")
    slbs = nc.dram_tensor("slbs", [P, NBANK * 2, 256], F16, kind="ExternalInput")
    slbk = nc.dram_tensor("slbk", [P, NBANK * 2, 256], F16, kind="ExternalInput")
    prm = nc.dram_tensor("prm", [P, 512], F16, kind="ExternalInput")
    drs = nc.dram_tensor("drs", [1, SH], F16, kind="ExternalInput")
    drk = nc.dram_tensor("drk", [1, SH], F16, kind="ExternalInput")
    w2 = nc.dram_tensor("w2", [P, 2, 40], F16, kind="ExternalInput")
    b1v = nc.dram_tensor("b1v", [P, 1], F32, kind="ExternalInput")
    h24 = nc.dram_tensor("h24", [P, SH // P, 64], F16, kind="ExternalOutput")
    plan_s["dram_s"], plan_s["dram_i"] = sa, ia
    plan_k["dram_s"], plan_k["dram_i"] = sk, ik
    with tile.TileContext(nc) as tc:
        with (
            tc.tile_pool(name="const", bufs=1) as cp,
            tc.tile_pool(name="gt", bufs=4) as pg,
            tc.tile_pool(name="stream", bufs=2) as pstream,
            tc.tile_pool(name="slab", bufs=2) as psl,
            tc.tile_pool(name="r1", bufs=2) as r1p,
            tc.tile_pool(name="ev", bufs=3) as ev,
            tc.tile_pool(name="ps", bufs=2, space="PSUM") as ps,
            tc.tile_pool(name="psb", bufs=1, space="PSUM") as psb,
            tc.tile_pool(name="ps2", bufs=2, space="PSUM") as ps2,
        ):
            w2t = cp.tile([P, 2, 40], F16)
            nc.sync.dma_start(out=w2t[:], in_=w2[:])
            b1t = cp.tile([P, 1], F32)
            nc.sync.dma_start(out=b1t[:], in_=b1v[:])
            prmt = cp.tile([P, 512], F16)
            nc.sync.dma_start(out=prmt[:], in_=prm[:])
            ones = cp.tile([1, P], F16)
            nc.vector.memset(ones[:], 1.0)

            es = AggEmitter(nc, (pg, pstream, psl, ps), plan_s, tbs, slbs, prmt, P, "s")
            ek = AggEmitter(nc, (pg, pstream, psl, ps), plan_k, tbk, slbk, prmt, P, "k")
            for b in range(NBANK):
                pa = es.emit_bank(b)
                pb = ek.emit_bank(b)
                drbs = r1p.tile([1, BANK], F16, tag="drbs")
                nc.sync.dma_start(out=drbs[:], in_=drs[:, b * 512:(b + 1) * 512])
                drbk = r1p.tile([1, BANK], F16, tag="drbk")
                nc.sync.dma_start(out=drbk[:], in_=drk[:, b * 512:(b + 1) * 512])
                bcs = psb.tile([P, BANK], F32, tag="bcs")
                nc.tensor.matmul(out=bcs[:], lhsT=ones[:], rhs=drbs[:],
                                 start=True, stop=True)
                bck = psb.tile([P, BANK], F32, tag="bck")
                nc.tensor.matmul(out=bck[:], lhsT=ones[:], rhs=drbk[:],
                                 start=True, stop=True)
                bcss = r1p.tile([P, BANK], F32, tag="bcss")
                bcks = r1p.tile([P, BANK], F32, tag="bcks")
                nc.vector.tensor_copy(bcss[:], bcs[:])
                nc.vector.tensor_copy(bcks[:], bck[:])
                za = r1p.tile([P, BANK], F32, tag="za")
                zb = r1p.tile([P, BANK], F32, tag="zb")
                nc.vector.tensor_mul(za[:], pa[:], bcss[:])
                nc.vector.tensor_mul(zb[:], pb[:], bcks[:])
                r1a = r1p.tile([P, BANK], F16, tag="r1a")
                r1b = r1p.tile([P, BANK], F16, tag="r1b")
                nc.scalar.activation(r1a[:], za[:], mybir.ActivationFunctionType.Relu,
                                     bias=b1t[:, :1], scale=1.0)
                nc.scalar.activation(r1b[:], zb[:], mybir.ActivationFunctionType.Relu,
                                     bias=b1t[:, :1], scale=1.0)
                he = ev.tile([P, 4, 64], F16, tag="he")
                nc.vector.memset(he[:], 0.0)
                for s in range(BANK // P):
                    pt = ps2.tile([P, 40], F32, tag="h2")
                    nc.tensor.matmul(out=pt[:], lhsT=r1a[:, s * P:(s + 1) * P],
                                     rhs=w2t[:, 0, :], start=True, stop=False)
                    nc.tensor.matmul(out=pt[:], lhsT=r1b[:, s * P:(s + 1) * P],
                                     rhs=w2t[:, 1, :], start=False, stop=True)
                    nc.vector.tensor_copy(he[:, s, 0:40], pt[:])
                nc.sync.dma_start(out=h24[:, b * 4:(b + 1) * 4, :], in_=he[:])
    nc.compile()
    return nc


# ------------------------------------------------------------- device: run3

def build_run3(plan_s, plan_k):
    """L2 aggregation (both graphs) + split Wlin matmuls + per-slot dinv_col
    scalars + folded bias + log_softmax for one core."""
    nc = bacc.Bacc(None, target_bir_lowering=False)
    tbs = nc.dram_tensor("tbs", [NPAD, P], F16, kind="ExternalInput")
    tbk = nc.dram_tensor("tbk", [NPAD, P], F16, kind="ExternalInput")
    sa = nc.dram_tensor("sa", [P, plan_s["n_stiles"] * STILE], F8, kind="ExternalInput")
    ia = nc.dram_tensor("ia", [P, plan_s["n_itiles"] * ITILE], I16, kind="ExternalInput")
    sk = nc.dram_tensor("sk", [P, plan_k["n_stiles"] * STILE], F8, kind="ExternalInput")
    ik = nc.dram_tensor("ik", [P, plan_k["n_itiles"] * ITILE], I16, kind="ExternalInput")
    slbs = nc.dram_tensor("slbs", [P, NBANK * 2, 256], F16, kind="ExternalInput")
    slbk = nc.dram_tensor("slbk", [P, NBANK * 2, 256], F16, kind="ExternalInput")
    prm = nc.dram_tensor("prm", [P, 512], F16, kind="ExternalInput")
    wl = nc.dram_tensor("wl", [40, 80], F16, kind="ExternalInput")
    dvs = nc.dram_tensor("dvs", [P, SH // P], F32, kind="ExternalInput")
    dvk = nc.dram_tensor("dvk", [P, SH // P], F32, kind="ExternalInput")
    bf = nc.dram_tensor("bf", [P, 40], F32, kind="ExternalInput")
    out4 = nc.dram_tensor("out4", [P, SH // P, 40], F32, kind="ExternalOutput")
    plan_s["dram_s"], plan_s["dram_i"] = sa, ia
    plan_k["dram_s"], plan_k["dram_i"] = sk, ik
    with tile.TileContext(nc) as tc:
        with (
            tc.tile_pool(name="const", bufs=1) as cp,
            tc.tile_pool(name="gt", bufs=4) as pg,
            tc.tile_pool(name="stream", bufs=2) as pstream,
            tc.tile_pool(name="slab", bufs=2) as psl,
            tc.tile_pool(name="r2", bufs=2) as r2p,
            tc.tile_pool(name="ev", bufs=4) as ev,
            tc.tile_pool(name="ps", bufs=2, space="PSUM") as ps,
            tc.tile_pool(name="ps2", bufs=2, space="PSUM") as ps2,
        ):
            wlt = cp.tile([40, 80], F16)
            nc.sync.dma_start(out=wlt[:], in_=wl[:])
            prmt = cp.tile([P, 512], F16)
            nc.sync.dma_start(out=prmt[:], in_=prm[:])
            dvst = cp.tile([P, SH // P], F32)
            nc.sync.dma_start(out=dvst[:], in_=dvs[:])
            dvkt = cp.tile([P, SH // P], F32)
            nc.sync.dma_start(out=dvkt[:], in_=dvk[:])
            bft = cp.tile([P, 40], F32)
            nc.sync.dma_start(out=bft[:], in_=bf[:])

            es = AggEmitter(nc, (pg, pstream, psl, ps), plan_s, tbs, slbs, prmt, 40, "s")
            ek = AggEmitter(nc, (pg, pstream, psl, ps), plan_k, tbk, slbk, prmt, 40, "k")
            for b in range(NBANK):
                pa = es.emit_bank(b)
                pb = ek.emit_bank(b)
                r2s = r2p.tile([40, BANK], F16, tag="r2s")
                r2k = r2p.tile([40, BANK], F16, tag="r2k")
                nc.vector.tensor_copy(r2s[:], pa[:40, :])
                nc.vector.tensor_copy(r2k[:], pb[:40, :])
                ot = ev.tile([P, 4, 40], F32, tag="ot")
                for s in range(BANK // P):
                    g = b * 4 + s
                    pts = ps2.tile([P, 40], F32, tag="lgs")
                    nc.tensor.matmul(out=pts[:], lhsT=r2s[:, s * P:(s + 1) * P],
                                     rhs=wlt[:, 0:40], start=True, stop=True)
                    ptk = ps2.tile([P, 40], F32, tag="lgk")
                    nc.tensor.matmul(out=ptk[:], lhsT=r2k[:, s * P:(s + 1) * P],
                                     rhs=wlt[:, 40:80], start=True, stop=True)
                    t1 = ev.tile([P, 40], F32, tag="t1")
                    nc.vector.tensor_scalar_mul(t1[:], pts[:], dvst[:, g:g + 1])
                    t2 = ev.tile([P, 40], F32, tag="t2")
                    nc.vector.tensor_scalar_mul(t2[:], ptk[:], dvkt[:, g:g + 1])
                    lg0 = ev.tile([P, 40], F32, tag="lg0")
                    nc.vector.tensor_add(lg0[:], t1[:], t2[:])
                    lg = ev.tile([P, 40], F32, tag="lg")
                    nc.vector.tensor_add(lg[:], lg0[:], bft[:])
                    mx = ev.tile([P, 1], F32, tag="mx")
                    nc.vector.tensor_reduce(mx[:], lg[:], mybir.AxisListType.X,
                                            mybir.AluOpType.max)
                    mxn = ev.tile([P, 1], F32, tag="mxn")
                    nc.vector.tensor_scalar_mul(mxn[:], mx[:], -1.0)
                    exm = ev.tile([P, 40], F32, tag="ex")
                    sm = ev.tile([P, 1], F32, tag="sm")
                    nc.scalar.activation(exm[:], lg[:], mybir.ActivationFunctionType.Exp,
                                         bias=mxn[:, :1], scale=1.0,
                                         accum_out=sm[:, :1])
                    ls = ev.tile([P, 1], F32, tag="ls")
                    nc.scalar.activation(ls[:], sm[:], mybir.ActivationFunctionType.Ln)
                    c = ev.tile([P, 1], F32, tag="c")
                    nc.vector.tensor_add(c[:], mx[:], ls[:])
                    nc.vector.tensor_scalar_sub(ot[:, s, :], lg[:], c[:, :1])
                nc.sync.dma_start(out=out4[:, b * 4:(b + 1) * 4, :], in_=ot[:])
    nc.compile()
    return nc


# ------------------------------------------------------------ device driver

class DeviceProgram:
    def __init__(self, nc, device):
        install_neuronx_cc_hook()
        self.nc = nc
        self.device = device
        partition_name = nc.partition_id_tensor.name if nc.partition_id_tensor else None
        in_names, out_names, out_avals, zero_outs = [], [], [], []
        for alloc in nc.m.functions[0].allocations:
            if not isinstance(alloc, mybir.MemoryLocationSet):
                continue
            name = alloc.memorylocations[0].name
            if alloc.kind == "ExternalInput":
                if name != partition_name:
                    in_names.append(name)
            elif alloc.kind == "ExternalOutput":
                shape = tuple(alloc.tensor_shape)
                dtype = mybir.dt.np(alloc.dtype)
                out_names.append(name)
                out_avals.append(jax.core.ShapedArray(shape, dtype))
                zero_outs.append(np.zeros(shape, dtype))
        self.in_names = list(in_names)
        self.out_names = out_names
        self.out_avals = out_avals
        self.zero_outs = zero_outs
        n_params = len(in_names)
        all_names = in_names + out_names + ([partition_name] if partition_name else [])
        self.n_params = n_params
        donate = tuple(range(n_params, n_params + len(out_names)))

        def _body(*args):
            operands = list(args)
            if partition_name is not None:
                operands.append(partition_id_tensor())
            outs = _bass_exec_p.bind(
                *operands,
                out_avals=tuple(out_avals),
                in_names=tuple(all_names),
                out_names=tuple(out_names),
                lowering_input_output_aliases=(),
                sim_require_finite=True,
                sim_require_nnan=True,
                nc=nc,
            )
            return tuple(outs)

        self.fn = jax.jit(_body, donate_argnums=donate, keep_unused=True)
        self.dev_inputs = None

    def upload(self, in_map):
        arrs = [np.asarray(in_map[n]) for n in self.in_names]
        self.dev_inputs = [jax.device_put(a, self.device) for a in arrs]

    def call(self):
        zo = [jax.device_put(z, self.device) for z in self.zero_outs]
        outs = self.fn(*self.dev_inputs, *zo)
        return outs

    def results(self, outs):
        return {n: np.asarray(o) for n, o in zip(self.out_names, outs)}


def _parallel(fns):
    outs = [None] * len(fns)
    errs = []

    def wrap(i):
        try:
            outs[i] = fns[i]()
        except Exception as e:  # noqa: BLE001
            import traceback
            errs.append((i, e, traceback.format_exc()))

    ts = [threading.Thread(target=wrap, args=(i,)) for i in range(len(fns))]
    for t in ts:
        t.start()
    for t in ts:
        t.join()
    if errs:
        raise RuntimeError(f"thread errors: {[(i, tb) for i, _, tb in errs]}")
    return outs


# ------------------------------------------------------------------ pipeline

def host_prep(edge_index, edge_index_knn):
    plans_s, plans_k = [], []
    for core in range(N_CORES):
        plans_s.append(pack_streams(build_shard_plan(edge_index, core)))
        plans_k.append(pack_streams(build_shard_plan(edge_index_knn, core)))
    return plans_s, plans_k


def build_programs(plans_s, plans_k, verbose=True):
    t0 = time.time()
    nc1 = build_run1()
    if verbose:
        print(f"[build] run1 {time.time()-t0:.1f}s", flush=True)
    nc2s, nc3s = [], []
    for core in range(N_CORES):
        t = time.time()
        nc2s.append(build_run2(plans_s[core], plans_k[core]))
        nc3s.append(build_run3(plans_s[core], plans_k[core]))
        if verbose:
            print(f"[build] core {core} run2+run3 {time.time()-t:.1f}s", flush=True)
    return nc1, nc2s, nc3s


def make_perm():
    pe = np.zeros((P, 512), np.float16)
    for i in range(P):
        pe[i, 2 * i] = 1.0          # P_even: row i -> col 2i
        pe[i, 256 + 2 * i + 1] = 1.0  # P_odd: row i -> col 2i+1
    return pe


def slab_of(tb, core):
    """Own-shard slab [128, NBANK*2, 256]: block j covers rows
    n0+j*256 .. +255; partition p holds rows (j*256+2p, j*256+2p+1)."""
    n0 = core * SH
    sl = tb[n0:n0 + SH].reshape(NBANK * 2, P, 256).transpose(1, 0, 2)
    return np.ascontiguousarray(sl)


class Pipeline:
    def __init__(self, inputs, verbose=True):
        self.v = verbose
        self.inputs = inputs
        self.devices = jax.devices()[:N_CORES]
        t0 = time.time()
        self.plans_s, self.plans_k = host_prep(
            inputs["edge_index"], inputs["edge_index_knn"])
        self.dinv_s = degrees_dinv(inputs["edge_index"])
        self.dinv_k = degrees_dinv(inputs["edge_index_knn"])
        if verbose:
            print(f"[prep] plans {time.time()-t0:.1f}s", flush=True)
        nc1, nc2s, nc3s = build_programs(self.plans_s, self.plans_k, verbose)
        t0 = time.time()
        self.p1 = [DeviceProgram(nc1, self.devices[i]) for i in range(N_CORES)]
        self.p2 = [DeviceProgram(nc2s[i], self.devices[i]) for i in range(N_CORES)]
        self.p3 = [DeviceProgram(nc3s[i], self.devices[i]) for i in range(N_CORES)]
        if verbose:
            print(f"[build] DevicePrograms {time.time()-t0:.1f}s", flush=True)
        self._prepare_inputs()

    def _prepare_inputs(self):
        ins = self.inputs
        x = np.asarray(ins["x"])
        W1 = np.asarray(ins["W1"]).astype(np.float16)
        W2 = np.asarray(ins["W2"]).astype(np.float16)
        Wlin = np.asarray(ins["Wlin"]).astype(np.float32)
        b1 = np.asarray(ins["b1"]).astype(np.float32)
        b2 = np.asarray(ins["b2"]).astype(np.float32)
        blin = np.asarray(ins["blin"]).astype(np.float32)

        w1p = np.zeros((512, P), np.float16)
        w1p[:500] = W1
        w14 = np.ascontiguousarray(w1p.reshape(4, P, P).transpose(1, 0, 2))
        self.run1_maps = []
        for i in range(N_CORES):
            xs = np.zeros((SH, 512), np.float16)
            lo, hi = i * SH, min((i + 1) * SH, N_REAL)
            if hi > lo:
                xs[:hi - lo, :500] = x[lo:hi].astype(np.float16)
            xT4 = np.ascontiguousarray(xs.T.reshape(4, P, SH).transpose(1, 0, 2))
            self.run1_maps.append({"xT4": xT4, "w14": w14})

        w2t = np.ascontiguousarray(
            np.asarray(W2).reshape(2, P, 40).transpose(1, 0, 2))
        self.consts2 = {"w2": w2t, "b1v": b1[:, None], "prm": make_perm()}
        WlT = Wlin.T  # [80, 40]
        wl = np.zeros((40, 80), np.float16)
        wl[:, 0:40] = WlT[0:40]
        wl[:, 40:80] = WlT[40:80]
        biasf = b2 @ (WlT[0:40] + WlT[40:80]).astype(np.float64) + blin
        bft = np.tile(biasf.astype(np.float32)[None, :], (P, 1))
        self.consts3 = {"wl": wl, "bf": bft, "prm": make_perm()}

        self.dr_s = [np.ascontiguousarray(
            self.dinv_s[i * SH:(i + 1) * SH].astype(np.float16).reshape(1, SH))
            for i in range(N_CORES)]
        self.dr_k = [np.ascontiguousarray(
            self.dinv_k[i * SH:(i + 1) * SH].astype(np.float16).reshape(1, SH))
            for i in range(N_CORES)]
        # per-slot-group [128, 100] fp32: dvs[p, g] = dinv[n0 + g*128 + p]
        self.dv_s = [np.ascontiguousarray(
            self.dinv_s[i * SH:(i + 1) * SH].reshape(SH // P, P).T)
            for i in range(N_CORES)]
        self.dv_k = [np.ascontiguousarray(
            self.dinv_k[i * SH:(i + 1) * SH].reshape(SH // P, P).T)
            for i in range(N_CORES)]

    def run(self):
        v = self.v
        t0 = time.time()
        for i in range(N_CORES):
            self.p1[i].upload(self.run1_maps[i])
        outs1 = _parallel([self.p1[i].call for i in range(N_CORES)])
        h_shards = [self.p1[i].results(outs1[i])["h4"] for i in range(N_CORES)]
        h_all = np.concatenate(
            [h.transpose(1, 0, 2).reshape(SH, P) for h in h_shards], axis=0)
        tb_s = (h_all.astype(np.float32) * self.dinv_s[:, None]).astype(np.float16)
        tb_k = (h_all.astype(np.float32) * self.dinv_k[:, None]).astype(np.float16)
        if v:
            print(f"[run1] done {time.time()-t0:.1f}s", flush=True)

        t0 = time.time()
        for i in range(N_CORES):
            m = {"tbs": tb_s, "tbk": tb_k,
                 "slbs": slab_of(tb_s, i), "slbk": slab_of(tb_k, i),
                 "sa": self.plans_s[i]["s_arr"], "ia": self.plans_s[i]["i_arr"],
                 "sk": self.plans_k[i]["s_arr"], "ik": self.plans_k[i]["i_arr"],
                 "drs": self.dr_s[i], "drk": self.dr_k[i],
                 **self.consts2}
            self.p2[i].upload(m)
        outs2 = _parallel([self.p2[i].call for i in range(N_CORES)])
        h2_shards = [self.p2[i].results(outs2[i])["h24"] for i in range(N_CORES)]
        h2_all = np.concatenate(
            [h.transpose(1, 0, 2).reshape(SH, 64) for h in h2_shards], axis=0)
        tb2_s = np.zeros((NPAD, P), np.float16)
        tb2_k = np.zeros((NPAD, P), np.float16)
        h2f = h2_all[:N_REAL, :40].astype(np.float32)
        tb2_s[:N_REAL, :40] = (h2f * self.dinv_s[:N_REAL, None]).astype(np.float16)
        tb2_k[:N_REAL, :40] = (h2f * self.dinv_k[:N_REAL, None]).astype(np.float16)
        if v:
            print(f"[run2] done {time.time()-t0:.1f}s", flush=True)

        t0 = time.time()
        for i in range(N_CORES):
            m = {"tbs": tb2_s, "tbk": tb2_k,
                 "slbs": slab_of(tb2_s, i), "slbk": slab_of(tb2_k, i),
                 "sa": self.plans_s[i]["s_arr"], "ia": self.plans_s[i]["i_arr"],
                 "sk": self.plans_k[i]["s_arr"], "ik": self.plans_k[i]["i_arr"],
                 "dvs": self.dv_s[i], "dvk": self.dv_k[i],
                 **self.consts3}
            self.p3[i].upload(m)
        outs3 = _parallel([self.p3[i].call for i in range(N_CORES)])
        out_shards = [self.p3[i].results(outs3[i])["out4"] for i in range(N_CORES)]
        result = np.concatenate(
            [o.transpose(1, 0, 2).reshape(SH, 40) for o in out_shards], axis=0)
        if v:
            print(f"[run3] done {time.time()-t0:.1f}s", flush=True)
        return result[:N_REAL]

    def time_runs(self, reps=5):
        times = {}
        for name, progs in (("run1", self.p1), ("run2", self.p2), ("run3", self.p3)):
            best = float("inf")
            for _ in range(reps):
                barrier = threading.Barrier(N_CORES + 1)
                done = []

                def worker(p):
                    barrier.wait()
                    o = p.call()
                    jax.block_until_ready(o)
                    done.append(o)

                ts = [threading.Thread(target=worker, args=(p,)) for p in progs]
                for t in ts:
                    t.start()
                barrier.wait()
                t0 = time.time()
                for t in ts:
                    t.join()
                best = min(best, time.time() - t0)
            times[name] = best
        return times


_PIPELINE_CACHE = {}


def _graph_key(inputs):
    ei = np.asarray(inputs["edge_index"])
    ek = np.asarray(inputs["edge_index_knn"])
    return (ei.shape, ek.shape, int(ei[:, 0].sum()), int(ei[:, -1].sum()),
            int(ek[:, 0].sum()), int(ek[:, -1].sum()))


def kernel(**inputs):
    key = "singleton"
    pl = _PIPELINE_CACHE.get(key)
    if pl is None or pl.graph_key != _graph_key(inputs):
        pl = Pipeline(inputs, verbose=False)
        pl.graph_key = _graph_key(inputs)
        _PIPELINE_CACHE[key] = pl
    else:
        pl.inputs = inputs
        pl._prepare_inputs()
    out = pl.run()
    return out.astype(np.float32)
